# revision 24
# baseline (speedup 1.0000x reference)
"""Trainium2 Bass kernel for nn_Decoder (pointer-generator style decoder step).

Strategy (8 NeuronCores, SPMD — identical program, per-core data):
  - Batch data-parallel: core c owns batches 8c..8c+8 (enc slice, sel/attn,
    score_c, final joint softmax + copy-scatter + log).
  - Tensor-parallel GRU/comb over hidden chunks (core c owns H rows
    128c..128c+128), with tiny AllGathers for gru_in / h_new assembly.
  - Tensor-parallel vocab projection: core c owns Wo rows 4000c..4000(c+1);
    an AllToAll redistributes score_g so each core holds full-vocab rows for
    its own batches.
  - All FLOPs on device. Host does only slicing / transposition / dtype
    packaging of inputs and reassembly of outputs.
"""

import os
import sys

import numpy as np

sys.path.insert(0, "/opt/trn_rl_repo")

import ml_dtypes  # noqa: E402

import concourse.bass as bass  # noqa: E402
import concourse.mybir as mybir  # noqa: E402
import concourse.tile as tile  # noqa: E402
from concourse import bacc  # noqa: E402

B = 64          # batch
L = 128         # max len
H = 1024        # hidden
E = 512         # embed
V = 32000       # vocab
MV = 33000      # max vocab (padded output)
NCORES = 8
BL = B // NCORES      # batches per core = 8
VC = V // NCORES      # vocab per core = 4000
HC = H // NCORES      # hidden chunk = 128
KV = 1536             # H + E
FOLD = MV // NCORES   # 4125, fold width for the log pass

F32 = mybir.dt.float32
BF16 = mybir.dt.bfloat16
F8 = mybir.dt.float8e4
I32 = mybir.dt.int32

RG = [list(range(NCORES))]


def _pt(pool, shape, dt, name):
    """Pool tile with a unique tag so loop-allocated tiles don't share slots."""
    return pool.tile(shape, dt, name=name, tag=name)


def build_nc():
    nc = bacc.Bacc("TRN2", target_bir_lowering=False, debug=False,
                   num_devices=NCORES)

    # ---------------- DRAM I/O ----------------
    enc_d = nc.dram_tensor("enc_own", [BL, L, H], F32, kind="ExternalInput")
    encT_d = nc.dram_tensor("encT_own", [BL, H, L], F32, kind="ExternalInput")
    wcT_d = nc.dram_tensor("wcT", [H, H], F32, kind="ExternalInput")
    wcb_d = nc.dram_tensor("wcb", [HC, NCORES], F32, kind="ExternalInput")
    woT_d = nc.dram_tensor("woT_own", [H, VC], F8, kind="ExternalInput")
    wob_d = nc.dram_tensor("wob_own", [1, VC], F8, kind="ExternalInput")
    wihT_d = nc.dram_tensor("wihT_own", [3, 2 * H, HC], BF16, kind="ExternalInput")
    whhT_d = nc.dram_tensor("whhT_own", [3, H, HC], F32, kind="ExternalInput")
    bih_d = nc.dram_tensor("bih_own", [3, HC, 1], F32, kind="ExternalInput")
    bhh_d = nc.dram_tensor("bhh_own", [3, HC, 1], F32, kind="ExternalInput")
    combT_d = nc.dram_tensor("combT_own", [KV, HC], BF16, kind="ExternalInput")
    combb_d = nc.dram_tensor("combb_own", [HC, 1], F32, kind="ExternalInput")
    attnWT_d = nc.dram_tensor("attnWT", [KV, L], F32, kind="ExternalInput")
    attnb_d = nc.dram_tensor("attnb", [1, L], F32, kind="ExternalInput")
    hT_d = nc.dram_tensor("hT", [H, B], F32, kind="ExternalInput")
    hTown_d = nc.dram_tensor("hT_own", [HC, B], F32, kind="ExternalInput")
    hTcols_d = nc.dram_tensor("hT_owncols", [H, BL], F32, kind="ExternalInput")
    emb_d = nc.dram_tensor("emb", [V, E], F32, kind="ExternalInput")
    tok64_d = nc.dram_tensor("tok64", [B, 1], I32, kind="ExternalInput")
    tokown_d = nc.dram_tensor("tok_own", [BL, 1], I32, kind="ExternalInput")
    seq_d = nc.dram_tensor("seq_own", [BL, L], I32, kind="ExternalInput")
    pre_d = nc.dram_tensor("pre_own", [BL, L], F32, kind="ExternalInput")
    ident_d = nc.dram_tensor("ident", [128, 128], F32, kind="ExternalInput")
    bc64_d = nc.dram_tensor("bc64", [BL, B], F32, kind="ExternalInput")
    selM_d = nc.dram_tensor("selM", [B, BL], F32, kind="ExternalInput")
    selB_d = nc.dram_tensor("selB", [B, BL], F32, kind="ExternalInput")
    onesf_d = nc.dram_tensor("onesf", [1, 128], F32, kind="ExternalInput")
    onesb_d = nc.dram_tensor("onesb", [1, B], F8, kind="ExternalInput")

    out1_d = nc.dram_tensor("out1_own", [BL, MV], F32, kind="ExternalOutput")
    attnw_d = nc.dram_tensor("attnw_own", [BL, L], F32, kind="ExternalOutput")
    probc_d = nc.dram_tensor("probc_own", [BL, L], F32, kind="ExternalOutput")
    hnewT_d = nc.dram_tensor("hnewT_own", [HC, B], F32, kind="ExternalOutput")


    with tile.TileContext(nc) as tc:
        with (
            tc.tile_pool(name="big", bufs=1) as big,       # resident sbuf
            tc.tile_pool(name="wrk", bufs=1) as wrk,       # misc sbuf
            tc.tile_pool(name="encj", bufs=2) as encj_p,   # streamed enc[b]
            tc.tile_pool(name="wct", bufs=8) as wct_p,    # streamed WcT tiles
            tc.tile_pool(name="ps_t", bufs=2, space="PSUM") as ps_t,    # small [128,128]
            tc.tile_pool(name="ps_s2", bufs=2, space="PSUM") as ps_s2,  # [128,512]
            tc.tile_pool(name="ps_gate", bufs=2, space="PSUM") as ps_gate,  # GRU
            tc.tile_pool(name="dram", bufs=1, space="DRAM") as dram,
        ):
            # ---------------- small loads ----------------
            ident = _pt(wrk, [128, 128], F32, "ident")
            nc.sync.dma_start(out=ident[:], in_=ident_d[:])
            bc64 = _pt(wrk, [BL, B], F32, "bc64")
            nc.sync.dma_start(out=bc64[:], in_=bc64_d[:])
            selM = _pt(wrk, [B, BL], F32, "selM")
            nc.sync.dma_start(out=selM[:], in_=selM_d[:])
            selB = _pt(wrk, [B, BL], F32, "selB")
            nc.sync.dma_start(out=selB[:], in_=selB_d[:])
            onesf = _pt(wrk, [1, 128], F32, "onesf")
            nc.sync.dma_start(out=onesf[:], in_=onesf_d[:])
            onesb = _pt(wrk, [1, B], F8, "onesb")
            nc.sync.dma_start(out=onesb[:], in_=onesb_d[:])
            wcb = _pt(wrk, [HC, NCORES], F32, "wcb")
            nc.sync.dma_start(out=wcb[:], in_=wcb_d[:])
            combb = _pt(wrk, [HC, 1], F32, "combb")
            nc.sync.dma_start(out=combb[:], in_=combb_d[:])
            attnb = _pt(wrk, [1, L], F32, "attnb")
            nc.sync.dma_start(out=attnb[:], in_=attnb_d[:])
            wob = _pt(wrk, [1, VC], F8, "wob")
            nc.sync.dma_start(out=wob[:], in_=wob_d[:])

            tok64 = _pt(wrk, [B, 1], I32, "tok64")
            nc.sync.dma_start(out=tok64[:], in_=tok64_d[:])
            tokown = _pt(wrk, [BL, 1], I32, "tokown")
            nc.sync.dma_start(out=tokown[:], in_=tokown_d[:])
            seqi = _pt(wrk, [BL, L], I32, "seqi")
            nc.sync.dma_start(out=seqi[:], in_=seq_d[:])
            pre = _pt(wrk, [BL, L], F32, "pre")
            nc.sync.dma_start(out=pre[:], in_=pre_d[:])

            bih = [_pt(wrk, [HC, 1], F32, f"bih{g}") for g in range(3)]
            bhh = [_pt(wrk, [HC, 1], F32, f"bhh{g}") for g in range(3)]
            for g in range(3):
                nc.sync.dma_start(out=bih[g][:], in_=bih_d[g])
                nc.sync.dma_start(out=bhh[g][:], in_=bhh_d[g])

            def load_rows(dram_ap, rows, cols, dt, pool, name):
                """Load a [rows, cols] DRAM tensor as rows//128 sbuf tiles."""
                n = rows // 128
                ts = []
                for k in range(n):
                    t = _pt(pool, [128, cols], dt, f"{name}{k}")
                    nc.sync.dma_start(out=t[:], in_=dram_ap[k * 128:(k + 1) * 128, :])
                    ts.append(t)
                return ts

            attnWT = load_rows(attnWT_d[:], KV, L, F32, big, "attnWT")
            combT = load_rows(combT_d[:], KV, HC, BF16, big, "combT")
            wihT = [load_rows(wihT_d[g], 2 * H, HC, BF16, big, f"wihT{g}_")
                    for g in range(3)]
            whhT = [load_rows(whhT_d[g], H, HC, F32, big, f"whhT{g}_")
                    for g in range(3)]
            hT = load_rows(hT_d[:], H, B, F32, big, "hT")
            hTown = _pt(wrk, [HC, B], F32, "hTown")
            nc.sync.dma_start(out=hTown[:], in_=hTown_d[:])
            hTcols = load_rows(hTcols_d[:], H, BL, F32, big, "hTcols")
            woT = load_rows(woT_d[:], H, VC, F8, big, "woT")

            # encT packed: tile k = [128 h, (b l)] bf16
            encTp = []
            for k in range(8):
                t = _pt(big, [128, BL * L], F32, f"encTp{k}")
                src = encT_d[:, k * 128:(k + 1) * 128, :].rearrange(
                    "b h l -> h b l")
                nc.sync.dma_start(out=t[:].rearrange("h (b l) -> h b l", b=BL),
                                  in_=src)
                encTp.append(t)

            identb = _pt(wrk, [128, 128], BF16, "identb")
            nc.vector.tensor_copy(out=identb[:], in_=ident[:])

            # ---------------- helpers ----------------
            def pe_T(in_ap, pin, pout, name, out_dt=F32):
                """PE transpose [pin, pout] -> sbuf [pout, pin]."""
                idt = ident if in_ap.dtype == F32 else identb
                ps = ps_t.tile([128, 128], in_ap.dtype, name=f"psT_{name}",
                               tag="psT")
                nc.tensor.transpose(out=ps[:pout, :pin], in_=in_ap,
                                    identity=idt[:pin, :pin])
                sb = _pt(wrk, [pout, pin], out_dt, f"T_{name}")
                nc.vector.tensor_copy(out=sb[:], in_=ps[:pout, :pin])
                return sb

            # ---------------- embedding gathers ----------------
            emb64 = _pt(big, [B, E], F32, "emb64")
            nc.gpsimd.indirect_dma_start(
                out=emb64[:], out_offset=None, in_=emb_d[:],
                in_offset=bass.IndirectOffsetOnAxis(ap=tok64[:, :1], axis=0))
            embown = _pt(wrk, [BL, E], F32, "embown")
            nc.gpsimd.indirect_dma_start(
                out=embown[:], out_offset=None, in_=emb_d[:],
                in_offset=bass.IndirectOffsetOnAxis(ap=tokown[:, :1], axis=0))

            embT64 = [pe_T(emb64[:, k * 128:(k + 1) * 128], B, 128,
                           f"embT64_{k}", out_dt=BF16) for k in range(4)]
            embTown = [pe_T(embown[:, k * 128:(k + 1) * 128], BL, 128,
                            f"embTown_{k}") for k in range(4)]

            # ---------------- attention scores (own batches) ----------------
            attn_lhs = embTown + [hTcols[k] for k in range(8)]
            ps_a = ps_t.tile([BL, L], F32, name="ps_a", tag="psT")
            for k in range(12):
                nc.tensor.matmul(out=ps_a[:], lhsT=attn_lhs[k][:],
                                 rhs=attnWT[k][:], start=(k == 0), stop=False)
            nc.tensor.matmul(out=ps_a[:], lhsT=onesf[:1, :BL], rhs=attnb[:],
                             start=False, stop=True)

            namax = _pt(wrk, [BL, 1], F32, "namax")
            nc.vector.reduce_max(out=namax[:], in_=ps_a[:],
                                 axis=mybir.AxisListType.X, negate=True)
            asum = _pt(wrk, [BL, 1], F32, "asum")
            nc.vector.memset(asum[:], 0.0)
            aexp = _pt(wrk, [BL, L], F32, "aexp")
            nc.scalar.activation(out=aexp[:], in_=ps_a[:],
                                 func=mybir.ActivationFunctionType.Exp,
                                 bias=namax[:, :1], accum_out=asum[:, :1])
            arec = _pt(wrk, [BL, 1], F32, "arec")
            nc.vector.reciprocal(out=arec[:], in_=asum[:])
            attnw = _pt(wrk, [BL, L], F32, "attnw")
            nc.vector.tensor_scalar_mul(attnw[:], aexp[:], arec[:, :1])
            nc.sync.dma_start(out=attnw_d[:], in_=attnw[:])
            attnwT = pe_T(attnw[:], BL, L, "attnwT")

            # ---------------- selective-read mask ----------------
            tokf = _pt(wrk, [BL, 1], F32, "tokf")
            nc.vector.tensor_copy(out=tokf[:], in_=tokown[:])
            seqf = _pt(wrk, [BL, L], F32, "seqf")
            nc.vector.tensor_copy(out=seqf[:], in_=seqi[:])
            eqm = _pt(wrk, [BL, L], F32, "eqm")
            nc.vector.tensor_scalar(eqm[:], seqf[:], tokf[:, :1], None,
                                    op0=mybir.AluOpType.is_equal)
            selv = _pt(wrk, [BL, L], F32, "selv")
            nc.vector.tensor_mul(out=selv[:], in0=eqm[:], in1=pre[:])
            selT = pe_T(selv[:], BL, L, "selT")

            # ---------------- per-batch einsums over enc ----------------
            # cc1_in[j, :H] = attn_applied[b=j]; [:, H:] = sel_reading[b=j]
            cc1_in = _pt(dram, [BL, 2 * H], BF16, "cc1_in")
            for j in range(BL):
                for h in range(2):
                    sl = slice(h * 512, (h + 1) * 512)
                    encb = encj_p.tile([L, 512], F32, name="encb", tag="encb")
                    nc.sync.dma_start(out=encb[:], in_=enc_d[j][:, sl])
                    enci = encj_p.tile([L, 512], I32, name="enci", tag="enci",
                                       bufs=1)
                    nc.vector.tensor_copy(out=enci[:], in_=encb[:])
                    enct = enci[:].bitcast(F32)
                    nc.vector.tensor_copy(out=enct, in_=enci[:])
                    psA = ps_t.tile([1, 512], F32, name="psA", tag="psT")
                    nc.tensor.matmul(out=psA[:],
                                     lhsT=attnwT[:, j:j + 1], rhs=encb[:],
                                     start=True, stop=True)
                    sbA = encj_p.tile([1, 512], BF16, name="sbA", tag="sbA",
                                      bufs=2)
                    nc.vector.tensor_copy(out=sbA[:], in_=psA[:])
                    nc.sync.dma_start(out=cc1_in[j:j + 1, sl], in_=sbA[:])
                    psS = ps_t.tile([1, 512], F32, name="psS", tag="psT")
                    nc.tensor.matmul(out=psS[:],
                                     lhsT=selT[:, j:j + 1], rhs=enct,
                                     start=True, stop=True, skip_group_check=True)
                    sbS = encj_p.tile([1, 512], BF16, name="sbS", tag="sbS",
                                      bufs=2)
                    nc.vector.tensor_copy(out=sbS[:], in_=psS[:])
                    nc.sync.dma_start(
                        out=cc1_in[j:j + 1, H + h * 512:H + (h + 1) * 512],
                        in_=sbS[:])

            # ---------------- AllGather 1: [attn_applied | sel_reading] ----
            cc1_out = dram.tile([B, 2 * H], BF16, name="cc1_out",
                                addr_space="Shared")
            nc.gpsimd.collective_compute(
                "AllGather", mybir.AluOpType.bypass, replica_groups=RG,
                ins=[cc1_in[:].opt()], outs=[cc1_out[:].opt()])

            rows1 = _pt(big, [B, 2 * H], BF16, "rows1")
            nc.sync.dma_start(out=rows1[:], in_=cc1_out[:])
            attnappT = [pe_T(rows1[:, k * 128:(k + 1) * 128], B, 128,
                             f"attnappT{k}", out_dt=BF16) for k in range(8)]
            selrT = [pe_T(rows1[:, H + k * 128:H + (k + 1) * 128], B, 128,
                          f"selrT{k}", out_dt=BF16) for k in range(8)]

            # ---------------- comb (TP over H chunk) ----------------
            catT = embT64 + attnappT
            ps_o = ps_t.tile([HC, B], F32, name="ps_o", tag="psT")
            for k in range(12):
                nc.tensor.matmul(out=ps_o[:], lhsT=combT[k][:], rhs=catT[k][:],
                                 start=(k == 0), stop=(k == 11))
            outT = _pt(wrk, [HC, B], BF16, "outT")
            nc.scalar.activation(out=outT[:], in_=ps_o[:],
                                 func=mybir.ActivationFunctionType.Relu,
                                 bias=combb[:, :1])

            cc2_in = _pt(dram, [HC, B], BF16, "cc2_in")
            nc.sync.dma_start(out=cc2_in[:], in_=outT[:])
            cc2_out = dram.tile([H, B], BF16, name="cc2_out",
                                addr_space="Shared")
            nc.gpsimd.collective_compute(
                "AllGather", mybir.AluOpType.bypass, replica_groups=RG,
                ins=[cc2_in[:].opt()], outs=[cc2_out[:].opt()])
            outTf = load_rows(cc2_out[:], H, B, BF16, big, "outTf")

            # ---------------- GRU (TP over H chunk) ----------------
            gruinT = outTf + selrT  # 16 tiles [128, 64]

            def gate_psum(g, wih_only, name):
                ps = ps_gate.tile([HC, B], F32, name=name, tag="gate")
                n_k = 16
                for k in range(n_k):
                    nc.tensor.matmul(out=ps[:], lhsT=wihT[g][k][:],
                                     rhs=gruinT[k][:], start=(k == 0),
                                     stop=(wih_only and k == n_k - 1))
                if not wih_only:
                    for k in range(8):
                        nc.tensor.matmul(out=ps[:], lhsT=whhT[g][k][:],
                                         rhs=hT[k][:], start=False,
                                         stop=(k == 7))
                return ps

            ps_hn = ps_gate.tile([HC, B], F32, name="ps_hn", tag="gate")
            for k in range(8):
                nc.tensor.matmul(out=ps_hn[:], lhsT=whhT[2][k][:], rhs=hT[k][:],
                                 start=(k == 0), stop=(k == 7))
            hn_g = _pt(wrk, [HC, B], F32, "hn_g")
            nc.vector.tensor_scalar_add(hn_g[:], ps_hn[:], bhh[2][:, :1])

            ps_r = gate_psum(0, False, "ps_r")
            brz = _pt(wrk, [HC, 1], F32, "brz")
            nc.vector.tensor_add(out=brz[:], in0=bih[0][:], in1=bhh[0][:])
            r_g = _pt(wrk, [HC, B], F32, "r_g")
            nc.scalar.activation(out=r_g[:], in_=ps_r[:],
                                 func=mybir.ActivationFunctionType.Sigmoid,
                                 bias=brz[:, :1])

            ps_in = gate_psum(2, True, "ps_in")
            ps_z = gate_psum(1, False, "ps_z")
            bzz = _pt(wrk, [HC, 1], F32, "bzz")
            nc.vector.tensor_add(out=bzz[:], in0=bih[1][:], in1=bhh[1][:])
            z_g = _pt(wrk, [HC, B], F32, "z_g")
            nc.scalar.activation(out=z_g[:], in_=ps_z[:],
                                 func=mybir.ActivationFunctionType.Sigmoid,
                                 bias=bzz[:, :1])
            rn = _pt(wrk, [HC, B], F32, "rn")
            nc.vector.tensor_mul(out=rn[:], in0=r_g[:], in1=hn_g[:])
            narg = _pt(wrk, [HC, B], F32, "narg")
            nc.vector.tensor_add(out=narg[:], in0=ps_in[:], in1=rn[:])
            n_g = _pt(wrk, [HC, B], F32, "n_g")
            nc.scalar.activation(out=n_g[:], in_=narg[:],
                                 func=mybir.ActivationFunctionType.Tanh,
                                 bias=bih[2][:, :1])
            dmn = _pt(wrk, [HC, B], F32, "dmn")
            nc.vector.tensor_sub(out=dmn[:], in0=hTown[:], in1=n_g[:])
            zd = _pt(wrk, [HC, B], F32, "zd")
            nc.vector.tensor_mul(out=zd[:], in0=z_g[:], in1=dmn[:])
            hnT_c = _pt(wrk, [HC, B], F32, "hnT_c")
            nc.vector.tensor_add(out=hnT_c[:], in0=n_g[:], in1=zd[:])
            nc.sync.dma_start(out=hnewT_d[:], in_=hnT_c[:])

            cc3_in = _pt(dram, [HC, B], F32, "cc3_in")
            nc.sync.dma_start(out=cc3_in[:], in_=hnT_c[:])
            cc3_out = dram.tile([H, B], F32, name="cc3_out",
                                addr_space="Shared")
            nc.gpsimd.collective_compute(
                "AllGather", mybir.AluOpType.bypass, replica_groups=RG,
                ins=[cc3_in[:].opt()], outs=[cc3_out[:].opt()])
            hnTf = load_rows(cc3_out[:], H, B, F32, big, "hnTf")

            hnTb = []
            for k in range(8):
                t = _pt(wrk, [128, B], F8, f"hnTb{k}")
                nc.vector.tensor_copy(out=t[:], in_=hnTf[k][:])
                hnTb.append(t)

            # own-batch h_new columns: rows -> select own -> transpose
            hrows = _pt(wrk, [B, H], F32, "hrows")
            for k in range(8):
                psh = ps_t.tile([128, 128], F32, name="psh", tag="psT")
                nc.tensor.transpose(out=psh[:B, :128], in_=hnTf[k][:],
                                    identity=ident[:, :])
                nc.vector.tensor_copy(out=hrows[:, k * 128:(k + 1) * 128],
                                      in_=psh[:B, :128])
            hrown = _pt(wrk, [BL, H], F32, "hrown")
            for h in range(2):
                psr = ps_s2.tile([128, 512], F32, name="psr", tag="ps2")
                nc.tensor.matmul(out=psr[:BL, :],
                                 lhsT=selB[:], rhs=hrows[:, h * 512:(h + 1) * 512],
                                 start=True, stop=True)
                nc.vector.tensor_copy(out=hrown[:, h * 512:(h + 1) * 512],
                                      in_=psr[:BL, :])
            hnTcol = [pe_T(hrown[:, k * 128:(k + 1) * 128], BL, 128,
                           f"hnTcol{k}") for k in range(8)]

            # ---------------- score_c: C1T + tanh + fused einsum ----------
            # per-ho partials in one PSUM tile; accumulate over ho in SBUF
            scacc = _pt(wrk, [L, BL], F32, "scacc")
            for ho in range(8):
                wcts = []
                for hi in range(8):
                    w = wct_p.tile([128, 128], F32, name="wct", tag="wct")
                    nc.sync.dma_start(
                        out=w[:],
                        in_=wcT_d[hi * 128:(hi + 1) * 128,
                                  ho * 128:(ho + 1) * 128])
                    wcts.append(w)
                t1 = wct_p.tile([128, BL * L], F32, name="t1", tag="t1",
                                bufs=2)
                for h in range(2):
                    psc = ps_s2.tile([128, 512], F32, name="psc", tag="ps2")
                    sl = slice(h * 512, (h + 1) * 512)
                    for hi in range(8):
                        nc.tensor.matmul(out=psc[:], lhsT=wcts[hi][:],
                                         rhs=encTp[hi][:, sl],
                                         start=(hi == 0), stop=(hi == 7))
                    nc.scalar.activation(out=t1[:, sl], in_=psc[:],
                                         func=mybir.ActivationFunctionType.Tanh,
                                         bias=wcb[:, ho:ho + 1])
                ps_sc = ps_gate.tile([L, BL], F32, name="ps_sc", tag="scT")
                for j in range(BL):
                    nc.tensor.matmul(out=ps_sc[:, j:j + 1],
                                     lhsT=t1[:, j * L:(j + 1) * L],
                                     rhs=hnTcol[ho][:, j:j + 1],
                                     start=(j == 0), stop=(j == BL - 1),
                                     skip_group_check=True)
                if ho == 0:
                    nc.vector.tensor_copy(out=scacc[:], in_=ps_sc[:])
                else:
                    nc.vector.tensor_add(out=scacc[:], in0=scacc[:],
                                         in1=ps_sc[:])
            scoreC = pe_T(scacc[:], L, BL, "scoreC")

            # ---------------- score_g (TP vocab) + AllToAll ----------------
            cc4_in = _pt(dram, [B, VC], F32, "cc4_in")
            NG = 500
            for v in range(VC // NG):
                psg = ps_s2.tile([B, NG], F32, name="psg", tag="ps2")
                sl = slice(v * NG, (v + 1) * NG)
                for k in range(8):
                    nc.tensor.matmul(out=psg[:], lhsT=hnTb[k][:],
                                     rhs=woT[k][:, sl], start=(k == 0),
                                     stop=False)
                nc.tensor.matmul(out=psg[:], lhsT=onesb[:1, :B], rhs=wob[:, sl],
                                 start=False, stop=True)
                sbg = encj_p.tile([B, NG], F32, name="sbg", tag="sbg", bufs=2)
                nc.vector.tensor_copy(out=sbg[:], in_=psg[:])
                nc.sync.dma_start(out=cc4_in[:, sl], in_=sbg[:])
            cc4_out = dram.tile([B, VC], F32, name="cc4_out")
            nc.gpsimd.collective_compute(
                "AllToAll", mybir.AluOpType.bypass, replica_groups=RG,
                ins=[cc4_in[:].opt()], outs=[cc4_out[:].opt()])

            # ---------------- joint softmax ----------------
            # SG partition p = 8*chunk + own-batch  (batch = p % 8)
            sgl = _pt(big, [B, FOLD], F32, "sgl")
            sg = sgl[:, :VC]
            nc.sync.dma_start(out=sg, in_=cc4_out[:])

            m2n = _pt(wrk, [BL, 1], F32, "m2n")
            nc.vector.reduce_max(out=m2n[:], in_=scoreC[:],
                                 axis=mybir.AxisListType.X, negate=True)
            s2 = _pt(wrk, [BL, 1], F32, "s2")
            nc.vector.memset(s2[:], 0.0)
            ec = _pt(wrk, [BL, L], F32, "ec")
            nc.scalar.activation(out=ec[:], in_=scoreC[:],
                                 func=mybir.ActivationFunctionType.Exp,
                                 bias=m2n[:, :1], accum_out=s2[:, :1])

            psb = ps_t.tile([B, 1], F32, name="psb", tag="psT")
            nc.tensor.matmul(out=psb[:], lhsT=bc64[:], rhs=m2n[:],
                             start=True, stop=True)
            m2n64 = _pt(wrk, [B, 1], F32, "m2n64")
            nc.vector.tensor_copy(out=m2n64[:], in_=psb[:])

            s1 = _pt(wrk, [B, 1], F32, "s1")
            nc.vector.memset(s1[:], 0.0)
            nc.scalar.activation(out=sg, in_=sg,
                                 func=mybir.ActivationFunctionType.Exp,
                                 bias=m2n64[:, :1], accum_out=s1[:, :1])

            psv = ps_t.tile([BL, 1], F32, name="psv", tag="psT")
            nc.tensor.matmul(out=psv[:BL, :], lhsT=selM[:], rhs=s1[:],
                             start=True, stop=True)
            stot = _pt(wrk, [BL, 1], F32, "stot")
            nc.vector.tensor_add(out=stot[:], in0=s2[:], in1=psv[:BL, :])
            rec = _pt(wrk, [BL, 1], F32, "rec")
            nc.vector.reciprocal(out=rec[:], in_=stot[:])

            probc = _pt(wrk, [BL, L], F32, "probc")
            nc.vector.tensor_scalar_mul(probc[:], ec[:], rec[:, :1])
            nc.sync.dma_start(out=probc_d[:], in_=probc[:])

            psb2 = ps_t.tile([B, 1], F32, name="psb2", tag="psT")
            nc.tensor.matmul(out=psb2[:], lhsT=bc64[:], rhs=rec[:],
                             start=True, stop=True)
            rec64 = _pt(wrk, [B, 1], F32, "rec64")
            nc.vector.tensor_copy(out=rec64[:], in_=psb2[:])
            nc.vector.tensor_scalar_mul(sg, sg, rec64[:, :1])

            # ---------------- scatter-add of copy probs ----------------
            rowb = [_pt(dram, [MV, 1], F32, f"rowb{j}") for j in range(BL)]
            padt = _pt(wrk, [BL, MV - V], F32, "padt")
            nc.vector.memset(padt[:], 1e-9)
            for j in range(BL):
                for k in range(8):
                    nc.sync.dma_start(
                        out=rowb[j][k * VC:(k + 1) * VC, 0:1],
                        in_=sgl[k * BL + j:k * BL + j + 1, :VC])
                nc.sync.dma_start(out=rowb[j][V:MV, 0:1],
                                  in_=padt[j:j + 1, :])

            seqfT = pe_T(seqf[:], BL, L, "seqfT")
            seqiT = _pt(wrk, [L, BL], I32, "seqiT")
            nc.vector.tensor_copy(out=seqiT[:], in_=seqfT[:])
            probcT = pe_T(probc[:], BL, L, "probcT")

            c2t = _pt(wrk, [BL, 1], F32, "c2t")
            nc.vector.memset(c2t[:], 1e-9)

            for j in range(BL):
                g_j = encj_p.tile([L, 1], F32, name="g_j", tag="g_j",
                                  bufs=2)
                nc.gpsimd.indirect_dma_start(
                    out=g_j[:], out_offset=None, in_=rowb[j][:],
                    in_offset=bass.IndirectOffsetOnAxis(
                        ap=seqiT[:, j:j + 1], axis=0))
                # selection matrix for duplicate indices within the row
                sri = encj_p.tile([1, L], I32, name="sri", tag="sri",
                                  bufs=2)
                nc.sync.dma_start(out=sri[:], in_=seq_d[j:j + 1, :])
                srf = encj_p.tile([1, L], F32, name="srf", tag="srf",
                                  bufs=2)
                nc.vector.tensor_copy(out=srf[:], in_=sri[:])
                psrep = ps_t.tile([L, L], F32, name="psrep", tag="psT")
                nc.tensor.matmul(out=psrep[:], lhsT=onesf[:1, :L],
                                 rhs=srf[:], start=True, stop=True)
                eqmat = encj_p.tile([L, L], F32, name="eqmat",
                                    tag="eqmat", bufs=1)
                nc.vector.tensor_scalar(eqmat[:], psrep[:],
                                        seqfT[:, j:j + 1], None,
                                        op0=mybir.AluOpType.is_equal)
                psdup = ps_t.tile([L, 1], F32, name="psdup", tag="psT")
                nc.tensor.matmul(out=psdup[:], lhsT=eqmat[:],
                                 rhs=probcT[:, j:j + 1], start=True, stop=True)
                val_j = encj_p.tile([L, 1], F32, name="val_j",
                                    tag="val_j", bufs=2)
                nc.vector.tensor_add(out=val_j[:], in0=g_j[:], in1=psdup[:])
                nc.gpsimd.indirect_dma_start(
                    out=rowb[j][:], out_offset=bass.IndirectOffsetOnAxis(
                        ap=seqiT[:, j:j + 1], axis=0),
                    in_=val_j[:], in_offset=None)
                nc.sync.dma_start(out=rowb[j][2:3, 0:1], in_=c2t[j:j + 1, :])

            # ---------------- log + store ----------------
            lg = sgl
            for j in range(BL):
                for k in range(8):
                    nc.sync.dma_start(
                        out=lg[k * BL + j:k * BL + j + 1, :],
                        in_=rowb[j][k * FOLD:(k + 1) * FOLD, 0:1])
            nc.scalar.activation(out=lg[:], in_=lg[:],
                                 func=mybir.ActivationFunctionType.Ln)
            for j in range(BL):
                for k in range(8):
                    nc.sync.dma_start(
                        out=out1_d[j:j + 1, k * FOLD:(k + 1) * FOLD],
                        in_=lg[k * BL + j:k * BL + j + 1, :])

    return nc


# ------------------------------------------------------------------
# host side
# ------------------------------------------------------------------
_NC_CACHE = {}


def _get_nc():
    if "nc" not in _NC_CACHE:
        _NC_CACHE["nc"] = build_nc()
    return _NC_CACHE["nc"]


def prepare_in_maps(inputs):
    f = lambda x: np.ascontiguousarray(np.asarray(x, dtype=np.float32))
    bf = lambda x: np.ascontiguousarray(
        np.asarray(x, dtype=np.float32).astype(ml_dtypes.bfloat16))
    f8 = lambda x: np.ascontiguousarray(
        np.asarray(x, dtype=np.float32).astype(ml_dtypes.float8_e4m3))
    i32 = lambda x: np.ascontiguousarray(np.asarray(x).astype(np.int32))

    enc = f(inputs["encoder_outputs"])          # [64, 128, 1024]
    h0 = f(inputs["hidden"])[0]                 # [64, 1024]
    emb = f(inputs["emb"])                      # [32000, 512]
    attn_W = f(inputs["attn_W"])                # [128, 1536]
    attn_b = f(inputs["attn_b"])                # [128]
    comb_W = f(inputs["comb_W"])                # [1024, 1536]
    comb_b = f(inputs["comb_b"])                # [1024]
    W_ih = f(inputs["W_ih"])                    # [3072, 2048]
    W_hh = f(inputs["W_hh"])                    # [3072, 1024]
    b_ih = f(inputs["b_ih"])                    # [3072]
    b_hh = f(inputs["b_hh"])                    # [3072]
    Wo_W = f(inputs["Wo_W"])                    # [32000, 1024]
    Wo_b = f(inputs["Wo_b"])                    # [32000]
    Wc_W = f(inputs["Wc_W"])                    # [1024, 1024]
    Wc_b = f(inputs["Wc_b"])                    # [1024]
    tok = i32(inputs["input_tok"]).reshape(B, 1)
    seq = i32(inputs["input_seq"])              # [64, 128]
    pre = f(inputs["pre_prob"])                 # [64, 128]

    hT = np.ascontiguousarray(h0.T)             # [1024, 64]
    wcT = np.ascontiguousarray(Wc_W.T)          # [1024, 1024]
    wcb = np.ascontiguousarray(Wc_b.reshape(8, HC).T)   # [128, 8]
    attnWT_f = np.ascontiguousarray(attn_W.T)
    ident = np.eye(128, dtype=np.float32)
    p_idx = np.arange(B)
    bc64 = (p_idx[None, :] % BL == np.arange(BL)[:, None]).astype(np.float32)
    selM = np.ascontiguousarray(bc64.T)
    onesf = np.ones((1, 128), np.float32)
    onesb = np.ones((1, B), np.float32).astype(ml_dtypes.float8_e4m3)

    in_maps = []
    for c in range(NCORES):
        bs = slice(c * BL, (c + 1) * BL)
        hs = slice(c * HC, (c + 1) * HC)
        vs = slice(c * VC, (c + 1) * VC)
        selB = np.zeros((B, BL), np.float32)
        selB[np.arange(c * BL, (c + 1) * BL), np.arange(BL)] = 1.0
        wihT = np.stack([
            np.ascontiguousarray(W_ih[g * H + c * HC:g * H + (c + 1) * HC, :].T)
            for g in range(3)]).astype(ml_dtypes.bfloat16)   # [3, 2048, 128]
        whhT = np.stack([
            np.ascontiguousarray(W_hh[g * H + c * HC:g * H + (c + 1) * HC, :].T)
            for g in range(3)])                              # [3, 1024, 128]
        bih = np.stack([b_ih[g * H + c * HC:g * H + (c + 1) * HC].reshape(HC, 1)
                        for g in range(3)])
        bhh = np.stack([b_hh[g * H + c * HC:g * H + (c + 1) * HC].reshape(HC, 1)
                        for g in range(3)])
        in_maps.append({
            "enc_own": np.ascontiguousarray(enc[bs]),
            "encT_own": np.ascontiguousarray(enc[bs].transpose(0, 2, 1)),
            "wcT": wcT,
            "wcb": wcb,
            "woT_own": f8(Wo_W[vs].T),
            "wob_own": f8(Wo_b[vs].reshape(1, VC)),
            "wihT_own": wihT,
            "whhT_own": whhT,
            "bih_own": bih,
            "bhh_own": bhh,
            "combT_own": bf(comb_W[hs].T),
            "combb_own": comb_b[hs].reshape(HC, 1).copy(),
            "attnWT": attnWT_f,
            "attnb": attn_b.reshape(1, L).copy(),
            "hT": hT,
            "hT_own": np.ascontiguousarray(hT[hs]),
            "hT_owncols": np.ascontiguousarray(hT[:, bs]),
            "emb": emb,
            "tok64": tok,
            "tok_own": np.ascontiguousarray(tok[bs]),
            "seq_own": np.ascontiguousarray(seq[bs]),
            "pre_own": np.ascontiguousarray(pre[bs]),
            "ident": ident,
            "bc64": bc64,
            "selM": selM,
            "selB": selB,
            "onesf": onesf,
            "onesb": onesb,
        })
    return in_maps


def assemble(results):
    out1 = np.concatenate([results[c]["out1_own"] for c in range(NCORES)], 0)
    attnw = np.concatenate([results[c]["attnw_own"] for c in range(NCORES)], 0)
    probc = np.concatenate([results[c]["probc_own"] for c in range(NCORES)], 0)
    hnew = np.concatenate(
        [results[c]["hnewT_own"].T for c in range(NCORES)], 1)[None]
    return (out1.astype(np.float32), hnew.astype(np.float32),
            attnw.astype(np.float32), probc.astype(np.float32))


def run_spmd(in_maps, trace=False):
    from concourse.bass_utils import run_bass_kernel_spmd
    nc = _get_nc()
    if not nc.is_finalized():
        nc.finalize()   # runs Bacc register allocation before serialization
    return run_bass_kernel_spmd(nc, in_maps, list(range(NCORES)), trace=trace)


def kernel(**inputs):
    in_maps = prepare_in_maps(inputs)
    res = run_spmd(in_maps)
    return assemble(res.results)


# revision 28
# speedup vs baseline: 1.3522x; 1.3522x over previous
"""Trainium2 Bass kernel for nn_Decoder (pointer-generator style decoder step).

Strategy (8 NeuronCores, SPMD — identical program, per-core data):
  - Batch data-parallel: core c owns batches 8c..8c+8 (enc slice, sel/attn,
    score_c, final joint softmax + copy-scatter + log).
  - Tensor-parallel GRU/comb over hidden chunks (core c owns H rows
    128c..128c+128), with tiny AllGathers for gru_in / h_new assembly.
  - Tensor-parallel vocab projection: core c owns Wo rows 4000c..4000(c+1);
    an AllToAll redistributes score_g so each core holds full-vocab rows for
    its own batches.
  - Precision: f32 on every path feeding score_c (logits reach +-30) and the
    h_new state; bf16 for small-magnitude weight matmuls (W_ih, comb); fp8
    for Wo (vocab scores only matter in log-domain).
  - DMA-instruction count is the scarce resource (HWDGE issue ~0.5us/inst):
    inputs are host-packed for big contiguous loads, bulk loads issue on the
    scalar ring, latency-critical loads on the sync ring.
"""

import sys

import numpy as np

sys.path.insert(0, "/opt/trn_rl_repo")

import ml_dtypes  # noqa: E402

import concourse.bass as bass  # noqa: E402
import concourse.mybir as mybir  # noqa: E402
import concourse.tile as tile  # noqa: E402
from concourse import bacc  # noqa: E402

B = 64          # batch
L = 128         # max len
H = 1024        # hidden
E = 512         # embed
V = 32000       # vocab
MV = 33000      # max vocab (padded output)
NCORES = 8
BL = B // NCORES      # batches per core = 8
VC = V // NCORES      # vocab per core = 4000
HC = H // NCORES      # hidden chunk = 128
KV = 1536             # H + E
FOLD = MV // NCORES   # 4125, fold width for the log pass

F32 = mybir.dt.float32
BF16 = mybir.dt.bfloat16
F8 = mybir.dt.float8e4
I32 = mybir.dt.int32

RG = [list(range(NCORES))]


def _pt(pool, shape, dt, name):
    return pool.tile(shape, dt, name=name, tag=name)


def build_nc(batched_indirect=False):
    nc = bacc.Bacc("TRN2", target_bir_lowering=False, debug=False,
                   num_devices=NCORES)

    # ---------------- DRAM I/O ----------------
    enc_d = nc.dram_tensor("enc_own", [BL, L, H], F32, kind="ExternalInput")
    encP_d = nc.dram_tensor("encP_own", [H, BL * L], F32, kind="ExternalInput")
    wcT_d = nc.dram_tensor("wcT", [H, H], F32, kind="ExternalInput")
    wcb_d = nc.dram_tensor("wcb", [HC, NCORES], F32, kind="ExternalInput")
    woT_d = nc.dram_tensor("woT_own", [H, VC], F8, kind="ExternalInput")
    wob_d = nc.dram_tensor("wob_own", [1, VC], F8, kind="ExternalInput")
    wihT_d = nc.dram_tensor("wihT3_own", [2 * H, 3 * HC], BF16,
                            kind="ExternalInput")
    whhT_d = nc.dram_tensor("whhT3_own", [H, 3 * HC], F32,
                            kind="ExternalInput")
    bih_d = nc.dram_tensor("bih_own", [3, HC, 1], F32, kind="ExternalInput")
    bhh_d = nc.dram_tensor("bhh_own", [3, HC, 1], F32, kind="ExternalInput")
    # columns: [attnWT (128) | combT chunk (128)] packed by K row, f32
    awc_d = nc.dram_tensor("awc_pack", [KV, 2 * L], F32, kind="ExternalInput")
    combb_d = nc.dram_tensor("combb_own", [HC, 1], F32, kind="ExternalInput")
    attnb_d = nc.dram_tensor("attnb", [1, L], F32, kind="ExternalInput")
    hT_d = nc.dram_tensor("hT", [H, B], F32, kind="ExternalInput")
    hTown_d = nc.dram_tensor("hT_own", [HC, B], F32, kind="ExternalInput")
    hTcols_d = nc.dram_tensor("hT_owncols", [H, BL], F32, kind="ExternalInput")
    emb_d = nc.dram_tensor("emb", [V, E], F32, kind="ExternalInput")
    tok64_d = nc.dram_tensor("tok64", [B, 1], I32, kind="ExternalInput")
    tokown_d = nc.dram_tensor("tok_own", [BL, 1], I32, kind="ExternalInput")
    seq_d = nc.dram_tensor("seq_own", [BL, L], I32, kind="ExternalInput")
    pre_d = nc.dram_tensor("pre_own", [BL, L], F32, kind="ExternalInput")
    ident_d = nc.dram_tensor("ident", [128, 128], F32, kind="ExternalInput")
    bc64_d = nc.dram_tensor("bc64", [BL, B], F32, kind="ExternalInput")
    selM_d = nc.dram_tensor("selM", [B, BL], F32, kind="ExternalInput")
    selB_d = nc.dram_tensor("selB", [B, BL], F32, kind="ExternalInput")
    onesf_d = nc.dram_tensor("onesf", [1, 128], F32, kind="ExternalInput")
    onesb_d = nc.dram_tensor("onesb", [1, B], F8, kind="ExternalInput")

    out1_d = nc.dram_tensor("out1_own", [BL, MV], F32, kind="ExternalOutput")
    attnw_d = nc.dram_tensor("attnw_own", [BL, L], F32, kind="ExternalOutput")
    probc_d = nc.dram_tensor("probc_own", [BL, L], F32, kind="ExternalOutput")
    hnewT_d = nc.dram_tensor("hnewT_own", [HC, B], F32, kind="ExternalOutput")

    with tile.TileContext(nc) as tc:
        with (
            tc.tile_pool(name="big", bufs=1) as big,
            tc.tile_pool(name="wrk", bufs=1) as wrk,
            tc.tile_pool(name="encj", bufs=2) as encj_p,
            tc.tile_pool(name="wct", bufs=4) as wct_p,
            tc.tile_pool(name="ps_t", bufs=2, space="PSUM") as ps_t,
            tc.tile_pool(name="ps_s2", bufs=2, space="PSUM") as ps_s2,
            tc.tile_pool(name="ps_gate", bufs=2, space="PSUM") as ps_gate,
            tc.tile_pool(name="dram", bufs=1, space="DRAM") as dram,
        ):
            # ============ phase 0: small latency-critical loads (sync) ====
            ident = _pt(wrk, [128, 128], F32, "ident")
            nc.sync.dma_start(out=ident[:], in_=ident_d[:])
            tok64 = _pt(wrk, [B, 1], I32, "tok64")
            nc.sync.dma_start(out=tok64[:], in_=tok64_d[:])
            tokown = _pt(wrk, [BL, 1], I32, "tokown")
            nc.sync.dma_start(out=tokown[:], in_=tokown_d[:])
            seqi = _pt(wrk, [BL, L], I32, "seqi")
            nc.sync.dma_start(out=seqi[:], in_=seq_d[:])
            pre = _pt(wrk, [BL, L], F32, "pre")
            nc.sync.dma_start(out=pre[:], in_=pre_d[:])

            def load_rows(dram_ap, rows, cols, dt, pool, name, eng):
                n = rows // 128
                ts = []
                for k in range(n):
                    t = _pt(pool, [128, cols], dt, f"{name}{k}")
                    eng.dma_start(out=t[:],
                                  in_=dram_ap[k * 128:(k + 1) * 128, :])
                    ts.append(t)
                return ts

            # attn weights + comb weights in one packed load (sync)
            awc = load_rows(awc_d[:], KV, 2 * L, F32, big, "awc", nc.sync)
            attnWT = [t[:, 0:L] for t in awc]
            hTcols = load_rows(hTcols_d[:], H, BL, F32, big, "hTcols",
                               nc.sync)
            attnb = _pt(wrk, [1, L], F32, "attnb")
            nc.sync.dma_start(out=attnb[:], in_=attnb_d[:])
            onesf = _pt(wrk, [1, 128], F32, "onesf")
            nc.sync.dma_start(out=onesf[:], in_=onesf_d[:])

            # ============ bulk loads on the scalar HWDGE ring =============
            woT = load_rows(woT_d[:], H, VC, F8, big, "woT", nc.scalar)
            encTp = load_rows(encP_d[:], H, BL * L, F32, big, "encTp",
                              nc.scalar)
            wihT3 = load_rows(wihT_d[:], 2 * H, 3 * HC, BF16, big, "wihT3",
                              nc.scalar)
            whhT3 = load_rows(whhT_d[:], H, 3 * HC, F32, big, "whhT3",
                              nc.scalar)
            hT = load_rows(hT_d[:], H, B, F32, big, "hT", nc.scalar)
            bc64 = _pt(wrk, [BL, B], F32, "bc64")
            nc.scalar.dma_start(out=bc64[:], in_=bc64_d[:])
            selM = _pt(wrk, [B, BL], F32, "selM")
            nc.scalar.dma_start(out=selM[:], in_=selM_d[:])
            selB = _pt(wrk, [B, BL], F32, "selB")
            nc.scalar.dma_start(out=selB[:], in_=selB_d[:])
            onesb = _pt(wrk, [1, B], F8, "onesb")
            nc.scalar.dma_start(out=onesb[:], in_=onesb_d[:])
            wcb = _pt(wrk, [HC, NCORES], F32, "wcb")
            nc.scalar.dma_start(out=wcb[:], in_=wcb_d[:])
            combb = _pt(wrk, [HC, 1], F32, "combb")
            nc.scalar.dma_start(out=combb[:], in_=combb_d[:])
            wob = _pt(wrk, [1, VC], F8, "wob")
            nc.scalar.dma_start(out=wob[:], in_=wob_d[:])
            hTown = _pt(wrk, [HC, B], F32, "hTown")
            nc.scalar.dma_start(out=hTown[:], in_=hTown_d[:])
            bih = [_pt(wrk, [HC, 1], F32, f"bih{g}") for g in range(3)]
            bhh = [_pt(wrk, [HC, 1], F32, f"bhh{g}") for g in range(3)]
            for g in range(3):
                nc.scalar.dma_start(out=bih[g][:], in_=bih_d[g])
                nc.scalar.dma_start(out=bhh[g][:], in_=bhh_d[g])

            # ============ helpers =========================================
            identb = _pt(wrk, [128, 128], BF16, "identb")
            nc.vector.tensor_copy(out=identb[:], in_=ident[:])

            def pe_T(in_ap, pin, pout, name, out_dt=F32):
                idt = ident if in_ap.dtype == F32 else identb
                ps = ps_t.tile([128, 128], in_ap.dtype, name=f"psT_{name}",
                               tag="psT")
                nc.tensor.transpose(out=ps[:pout, :pin], in_=in_ap,
                                    identity=idt[:pin, :pin])
                sb = _pt(wrk, [pout, pin], out_dt, f"T_{name}")
                nc.vector.tensor_copy(out=sb[:], in_=ps[:pout, :pin])
                return sb

            # ============ embedding gathers ===============================
            emb64 = _pt(big, [B, E], F32, "emb64")
            nc.gpsimd.indirect_dma_start(
                out=emb64[:], out_offset=None, in_=emb_d[:],
                in_offset=bass.IndirectOffsetOnAxis(ap=tok64[:, :1], axis=0))
            embown = _pt(wrk, [BL, E], F32, "embown")
            nc.gpsimd.indirect_dma_start(
                out=embown[:], out_offset=None, in_=emb_d[:],
                in_offset=bass.IndirectOffsetOnAxis(ap=tokown[:, :1], axis=0))

            embT64 = [pe_T(emb64[:, k * 128:(k + 1) * 128], B, 128,
                           f"embT64_{k}", out_dt=BF16) for k in range(4)]
            embTown = [pe_T(embown[:, k * 128:(k + 1) * 128], BL, 128,
                            f"embTown_{k}") for k in range(4)]

            # ============ attention scores (own batches) ==================
            attn_lhs = embTown + [hTcols[k] for k in range(8)]
            ps_a = ps_t.tile([BL, L], F32, name="ps_a", tag="psT")
            for k in range(12):
                nc.tensor.matmul(out=ps_a[:], lhsT=attn_lhs[k][:],
                                 rhs=attnWT[k], start=(k == 0), stop=False)
            nc.tensor.matmul(out=ps_a[:], lhsT=onesf[:1, :BL], rhs=attnb[:],
                             start=False, stop=True)

            namax = _pt(wrk, [BL, 1], F32, "namax")
            nc.vector.reduce_max(out=namax[:], in_=ps_a[:],
                                 axis=mybir.AxisListType.X, negate=True)
            asum = _pt(wrk, [BL, 1], F32, "asum")
            nc.vector.memset(asum[:], 0.0)
            aexp = _pt(wrk, [BL, L], F32, "aexp")
            nc.scalar.activation(out=aexp[:], in_=ps_a[:],
                                 func=mybir.ActivationFunctionType.Exp,
                                 bias=namax[:, :1], accum_out=asum[:, :1])
            arec = _pt(wrk, [BL, 1], F32, "arec")
            nc.vector.reciprocal(out=arec[:], in_=asum[:])
            attnw = _pt(wrk, [BL, L], F32, "attnw")
            nc.vector.tensor_scalar_mul(attnw[:], aexp[:], arec[:, :1])
            nc.sync.dma_start(out=attnw_d[:], in_=attnw[:])
            attnwT = pe_T(attnw[:], BL, L, "attnwT")

            # ============ selective-read mask =============================
            tokf = _pt(wrk, [BL, 1], F32, "tokf")
            nc.vector.tensor_copy(out=tokf[:], in_=tokown[:])
            seqf = _pt(wrk, [BL, L], F32, "seqf")
            nc.vector.tensor_copy(out=seqf[:], in_=seqi[:])
            eqm = _pt(wrk, [BL, L], F32, "eqm")
            nc.vector.tensor_scalar(eqm[:], seqf[:], tokf[:, :1], None,
                                    op0=mybir.AluOpType.is_equal)
            selv = _pt(wrk, [BL, L], F32, "selv")
            nc.vector.tensor_mul(out=selv[:], in0=eqm[:], in1=pre[:])
            selT = pe_T(selv[:], BL, L, "selT")

            # ============ per-batch einsums over enc ======================
            cc1_in = _pt(dram, [BL, 2 * H], BF16, "cc1_in")
            for j in range(BL):
                row = encj_p.tile([1, 2 * H], BF16, name="ccrow", tag="ccrow",
                                  bufs=2)
                for h in range(2):
                    sl = slice(h * 512, (h + 1) * 512)
                    encb = encj_p.tile([L, 512], F32, name="encb", tag="encb")
                    nc.sync.dma_start(out=encb[:], in_=enc_d[j][:, sl])
                    enci = encj_p.tile([L, 512], I32, name="enci", tag="enci",
                                       bufs=2)
                    nc.vector.tensor_copy(out=enci[:], in_=encb[:])
                    enct = enci[:].bitcast(F32)
                    nc.vector.tensor_copy(out=enct, in_=enci[:])
                    psA = ps_t.tile([1, 512], F32, name="psA", tag="psT")
                    nc.tensor.matmul(out=psA[:], lhsT=attnwT[:, j:j + 1],
                                     rhs=encb[:], start=True, stop=True)
                    nc.vector.tensor_copy(out=row[:, sl], in_=psA[:])
                    psS = ps_t.tile([1, 512], F32, name="psS", tag="psT")
                    nc.tensor.matmul(out=psS[:], lhsT=selT[:, j:j + 1],
                                     rhs=enct, start=True, stop=True,
                                     skip_group_check=True)
                    nc.vector.tensor_copy(
                        out=row[:, H + h * 512:H + (h + 1) * 512], in_=psS[:])
                nc.sync.dma_start(out=cc1_in[j:j + 1, :], in_=row[:])

            # ============ AllGather 1: [attn_applied | sel_reading] =======
            cc1_out = dram.tile([B, 2 * H], BF16, name="cc1_out",
                                addr_space="Shared")
            nc.gpsimd.collective_compute(
                "AllGather", mybir.AluOpType.bypass, replica_groups=RG,
                ins=[cc1_in[:].opt()], outs=[cc1_out[:].opt()])

            rows1 = _pt(big, [B, 2 * H], BF16, "rows1")
            nc.sync.dma_start(out=rows1[:], in_=cc1_out[:])
            attnappT = [pe_T(rows1[:, k * 128:(k + 1) * 128], B, 128,
                             f"attnappT{k}", out_dt=BF16) for k in range(8)]
            selrT = [pe_T(rows1[:, H + k * 128:H + (k + 1) * 128], B, 128,
                          f"selrT{k}", out_dt=BF16) for k in range(8)]

            # ============ comb (TP over H chunk) ==========================
            combTb = []
            for k in range(12):
                t = _pt(wrk, [128, L], BF16, f"combTb{k}")
                nc.vector.tensor_copy(out=t[:], in_=awc[k][:, L:2 * L])
                combTb.append(t)
            catT = embT64 + attnappT
            ps_o = ps_t.tile([HC, B], F32, name="ps_o", tag="psT")
            for k in range(12):
                nc.tensor.matmul(out=ps_o[:], lhsT=combTb[k][:, :HC],
                                 rhs=catT[k][:], start=(k == 0),
                                 stop=(k == 11))
            outT = _pt(wrk, [HC, B], BF16, "outT")
            nc.scalar.activation(out=outT[:], in_=ps_o[:],
                                 func=mybir.ActivationFunctionType.Relu,
                                 bias=combb[:, :1])

            cc2_in = _pt(dram, [HC, B], BF16, "cc2_in")
            nc.sync.dma_start(out=cc2_in[:], in_=outT[:])
            cc2_out = dram.tile([H, B], BF16, name="cc2_out",
                                addr_space="Shared")
            nc.gpsimd.collective_compute(
                "AllGather", mybir.AluOpType.bypass, replica_groups=RG,
                ins=[cc2_in[:].opt()], outs=[cc2_out[:].opt()])
            outTf = load_rows(cc2_out[:], H, B, BF16, big, "outTf", nc.sync)

            # ============ GRU (TP over H chunk) ===========================
            gruinT = outTf + selrT  # 16 bf16 tiles [128, 64]

            def gate_bf(g, ps, stop_last):
                for k in range(16):
                    nc.tensor.matmul(out=ps[:],
                                     lhsT=wihT3[k][:, g * HC:(g + 1) * HC],
                                     rhs=gruinT[k][:], start=(k == 0),
                                     stop=(stop_last and k == 15),
                                     skip_group_check=True)

            def gate_f32(g, ps, start, stop):
                for k in range(8):
                    nc.tensor.matmul(out=ps[:],
                                     lhsT=whhT3[k][:, g * HC:(g + 1) * HC],
                                     rhs=hT[k][:], start=(start and k == 0),
                                     stop=(stop and k == 7),
                                     skip_group_check=True)

            ps_hn = ps_gate.tile([HC, B], F32, name="ps_hn", tag="gate")
            gate_f32(2, ps_hn, True, True)
            hn_g = _pt(wrk, [HC, B], F32, "hn_g")
            nc.vector.tensor_scalar_add(hn_g[:], ps_hn[:], bhh[2][:, :1])

            ps_r = ps_gate.tile([HC, B], F32, name="ps_r", tag="gate")
            gate_bf(0, ps_r, False)
            gate_f32(0, ps_r, False, True)
            brz = _pt(wrk, [HC, 1], F32, "brz")
            nc.vector.tensor_add(out=brz[:], in0=bih[0][:], in1=bhh[0][:])
            r_g = _pt(wrk, [HC, B], F32, "r_g")
            nc.scalar.activation(out=r_g[:], in_=ps_r[:],
                                 func=mybir.ActivationFunctionType.Sigmoid,
                                 bias=brz[:, :1])

            ps_in = ps_gate.tile([HC, B], F32, name="ps_in", tag="gate")
            gate_bf(2, ps_in, True)
            ps_z = ps_gate.tile([HC, B], F32, name="ps_z", tag="gate")
            gate_bf(1, ps_z, False)
            gate_f32(1, ps_z, False, True)
            bzz = _pt(wrk, [HC, 1], F32, "bzz")
            nc.vector.tensor_add(out=bzz[:], in0=bih[1][:], in1=bhh[1][:])
            z_g = _pt(wrk, [HC, B], F32, "z_g")
            nc.scalar.activation(out=z_g[:], in_=ps_z[:],
                                 func=mybir.ActivationFunctionType.Sigmoid,
                                 bias=bzz[:, :1])

            rn = _pt(wrk, [HC, B], F32, "rn")
            nc.vector.tensor_mul(out=rn[:], in0=r_g[:], in1=hn_g[:])
            narg = _pt(wrk, [HC, B], F32, "narg")
            nc.vector.tensor_add(out=narg[:], in0=ps_in[:], in1=rn[:])
            n_g = _pt(wrk, [HC, B], F32, "n_g")
            nc.scalar.activation(out=n_g[:], in_=narg[:],
                                 func=mybir.ActivationFunctionType.Tanh,
                                 bias=bih[2][:, :1])
            dmn = _pt(wrk, [HC, B], F32, "dmn")
            nc.vector.tensor_sub(out=dmn[:], in0=hTown[:], in1=n_g[:])
            zd = _pt(wrk, [HC, B], F32, "zd")
            nc.vector.tensor_mul(out=zd[:], in0=z_g[:], in1=dmn[:])
            hnT_c = _pt(wrk, [HC, B], F32, "hnT_c")
            nc.vector.tensor_add(out=hnT_c[:], in0=n_g[:], in1=zd[:])
            nc.sync.dma_start(out=hnewT_d[:], in_=hnT_c[:])

            cc3_in = _pt(dram, [HC, B], F32, "cc3_in")
            nc.sync.dma_start(out=cc3_in[:], in_=hnT_c[:])
            cc3_out = dram.tile([H, B], F32, name="cc3_out",
                                addr_space="Shared")
            nc.gpsimd.collective_compute(
                "AllGather", mybir.AluOpType.bypass, replica_groups=RG,
                ins=[cc3_in[:].opt()], outs=[cc3_out[:].opt()])
            hnTf = load_rows(cc3_out[:], H, B, F32, big, "hnTf", nc.sync)

            # ============ score_g (fp8 TP vocab) + AllToAll ===============
            hnT8 = []
            for k in range(8):
                t = _pt(wrk, [128, B], F8, f"hnT8{k}")
                nc.vector.tensor_copy(out=t[:], in_=hnTf[k][:])
                hnT8.append(t)
            cc4_in = _pt(dram, [B, VC], F32, "cc4_in")
            NG = 500
            for v in range(VC // NG):
                psg = ps_s2.tile([B, NG], F32, name="psg", tag="ps2")
                sl = slice(v * NG, (v + 1) * NG)
                for k in range(8):
                    nc.tensor.matmul(out=psg[:], lhsT=hnT8[k][:],
                                     rhs=woT[k][:, sl], start=(k == 0),
                                     stop=False)
                nc.tensor.matmul(out=psg[:], lhsT=onesb[:1, :B],
                                 rhs=wob[:, sl], start=False, stop=True)
                sbg = encj_p.tile([B, NG], F32, name="sbg", tag="sbg",
                                  bufs=2)
                nc.vector.tensor_copy(out=sbg[:], in_=psg[:])
                nc.sync.dma_start(out=cc4_in[:, sl], in_=sbg[:])
            cc4_out = dram.tile([B, VC], F32, name="cc4_out")
            nc.gpsimd.collective_compute(
                "AllToAll", mybir.AluOpType.bypass, replica_groups=RG,
                ins=[cc4_in[:].opt()], outs=[cc4_out[:].opt()])

            # ============ own-batch h_new columns =========================
            hrows = _pt(wrk, [B, H], F32, "hrows")
            for k in range(8):
                psh = ps_t.tile([128, 128], F32, name="psh", tag="psT")
                nc.tensor.transpose(out=psh[:B, :128], in_=hnTf[k][:],
                                    identity=ident[:, :])
                nc.vector.tensor_copy(out=hrows[:, k * 128:(k + 1) * 128],
                                      in_=psh[:B, :128])
            hrown = _pt(wrk, [BL, H], F32, "hrown")
            for h in range(2):
                psr = ps_s2.tile([128, 512], F32, name="psr", tag="ps2")
                nc.tensor.matmul(out=psr[:BL, :], lhsT=selB[:],
                                 rhs=hrows[:, h * 512:(h + 1) * 512],
                                 start=True, stop=True)
                nc.vector.tensor_copy(out=hrown[:, h * 512:(h + 1) * 512],
                                      in_=psr[:BL, :])
            hnTcol = [pe_T(hrown[:, k * 128:(k + 1) * 128], BL, 128,
                           f"hnTcol{k}") for k in range(8)]

            # ============ score_c: C1T + tanh + fused einsum ==============
            scacc = _pt(wrk, [L, BL], F32, "scacc")
            for hop in range(4):          # pairs of ho chunks
                wcts = []
                for hi in range(8):
                    w = wct_p.tile([128, 256], F32, name="wct", tag="wct",
                                   bufs=8)
                    nc.sync.dma_start(
                        out=w[:], in_=wcT_d[hi * 128:(hi + 1) * 128,
                                           hop * 256:(hop + 1) * 256])
                    wcts.append(w)
                for hoi in range(2):
                    ho = hop * 2 + hoi
                    t1 = wct_p.tile([128, BL * L], F32, name="t1", tag="t1",
                                    bufs=2)
                    for h in range(2):
                        psc = ps_s2.tile([128, 512], F32, name="psc",
                                         tag="ps2")
                        sl = slice(h * 512, (h + 1) * 512)
                        for hi in range(8):
                            nc.tensor.matmul(
                                out=psc[:],
                                lhsT=wcts[hi][:, hoi * 128:(hoi + 1) * 128],
                                rhs=encTp[hi][:, sl],
                                start=(hi == 0), stop=(hi == 7))
                        nc.scalar.activation(
                            out=t1[:, sl], in_=psc[:],
                            func=mybir.ActivationFunctionType.Tanh,
                            bias=wcb[:, ho:ho + 1])
                    ps_sc = ps_gate.tile([L, BL], F32, name="ps_sc",
                                         tag="scT")
                    for j in range(BL):
                        nc.tensor.matmul(out=ps_sc[:, j:j + 1],
                                         lhsT=t1[:, j * L:(j + 1) * L],
                                         rhs=hnTcol[ho][:, j:j + 1],
                                         start=(j == 0), stop=(j == BL - 1),
                                         skip_group_check=True)
                    if ho == 0:
                        nc.vector.tensor_copy(out=scacc[:], in_=ps_sc[:])
                    else:
                        nc.vector.tensor_add(out=scacc[:], in0=scacc[:],
                                             in1=ps_sc[:])
            scoreC = pe_T(scacc[:], L, BL, "scoreC")

            # ============ joint softmax ===================================
            # sgl partition p = 8*chunk + own-batch (batch = p % 8)
            sgl = _pt(big, [B, FOLD], F32, "sgl")
            sg = sgl[:, :VC]
            nc.sync.dma_start(out=sg, in_=cc4_out[:])

            m2n = _pt(wrk, [BL, 1], F32, "m2n")
            nc.vector.reduce_max(out=m2n[:], in_=scoreC[:],
                                 axis=mybir.AxisListType.X, negate=True)
            s2 = _pt(wrk, [BL, 1], F32, "s2")
            nc.vector.memset(s2[:], 0.0)
            ec = _pt(wrk, [BL, L], F32, "ec")
            nc.scalar.activation(out=ec[:], in_=scoreC[:],
                                 func=mybir.ActivationFunctionType.Exp,
                                 bias=m2n[:, :1], accum_out=s2[:, :1])

            psb = ps_t.tile([B, 1], F32, name="psb", tag="psT")
            nc.tensor.matmul(out=psb[:], lhsT=bc64[:], rhs=m2n[:],
                             start=True, stop=True)
            m2n64 = _pt(wrk, [B, 1], F32, "m2n64")
            nc.vector.tensor_copy(out=m2n64[:], in_=psb[:])

            s1 = _pt(wrk, [B, 1], F32, "s1")
            nc.vector.memset(s1[:], 0.0)
            nc.scalar.activation(out=sg, in_=sg,
                                 func=mybir.ActivationFunctionType.Exp,
                                 bias=m2n64[:, :1], accum_out=s1[:, :1])

            psv = ps_t.tile([BL, 1], F32, name="psv", tag="psT")
            nc.tensor.matmul(out=psv[:], lhsT=selM[:], rhs=s1[:],
                             start=True, stop=True)
            stot = _pt(wrk, [BL, 1], F32, "stot")
            nc.vector.tensor_add(out=stot[:], in0=s2[:], in1=psv[:])
            rec = _pt(wrk, [BL, 1], F32, "rec")
            nc.vector.reciprocal(out=rec[:], in_=stot[:])

            probc = _pt(wrk, [BL, L], F32, "probc")
            nc.vector.tensor_scalar_mul(probc[:], ec[:], rec[:, :1])
            nc.sync.dma_start(out=probc_d[:], in_=probc[:])

            psb2 = ps_t.tile([B, 1], F32, name="psb2", tag="psT")
            nc.tensor.matmul(out=psb2[:], lhsT=bc64[:], rhs=rec[:],
                             start=True, stop=True)
            rec64 = _pt(wrk, [B, 1], F32, "rec64")
            nc.vector.tensor_copy(out=rec64[:], in_=psb2[:])
            nc.vector.tensor_scalar_mul(sg, sg, rec64[:, :1])

            # ============ scatter-add of copy probs =======================
            rowb = _pt(dram, [BL, MV], F32, "rowb")
            rowflat = rowb[:].rearrange("a b -> (a b)")[:, None]
            for k in range(8):
                nc.sync.dma_start(
                    out=rowb[0:BL, k * VC:(k + 1) * VC],
                    in_=sgl[k * BL:(k + 1) * BL, :VC])
            padt = _pt(wrk, [BL, MV - V], F32, "padt")
            nc.vector.memset(padt[:], 1e-9)
            nc.sync.dma_start(out=rowb[0:BL, V:MV], in_=padt[:])

            seqfT = pe_T(seqf[:], BL, L, "seqfT")
            seqiT = _pt(wrk, [L, BL], I32, "seqiT")
            nc.vector.tensor_copy(out=seqiT[:], in_=seqfT[:])
            ioff = _pt(wrk, [L, BL], I32, "ioff")
            nc.gpsimd.iota(ioff[:], pattern=[[1, BL]], base=0,
                           channel_multiplier=0)
            ioffm = _pt(wrk, [L, BL], I32, "ioffm")
            nc.vector.tensor_scalar(ioffm[:], ioff[:], MV, None,
                                    op0=mybir.AluOpType.mult)
            soff = _pt(wrk, [L, BL], I32, "soff")
            nc.vector.tensor_add(out=soff[:], in0=seqiT[:], in1=ioffm[:])
            probcT = pe_T(probc[:], BL, L, "probcT")

            # duplicate-index accumulation (selmat per batch)
            vals = _pt(wrk, [L, BL], F32, "vals")
            for j in range(BL):
                sri = encj_p.tile([1, L], I32, name="sri", tag="sri", bufs=2)
                nc.sync.dma_start(out=sri[:], in_=seq_d[j:j + 1, :])
                srf = encj_p.tile([1, L], F32, name="srf", tag="srf", bufs=2)
                nc.vector.tensor_copy(out=srf[:], in_=sri[:])
                psrep = ps_t.tile([L, L], F32, name="psrep", tag="psT")
                nc.tensor.matmul(out=psrep[:], lhsT=onesf[:1, :L],
                                 rhs=srf[:], start=True, stop=True)
                eqmat = encj_p.tile([L, L], F32, name="eqmat", tag="eqmat",
                                    bufs=1)
                nc.vector.tensor_scalar(eqmat[:], psrep[:],
                                        seqfT[:, j:j + 1], None,
                                        op0=mybir.AluOpType.is_equal)
                psdup = ps_t.tile([L, 1], F32, name="psdup", tag="psT")
                nc.tensor.matmul(out=psdup[:], lhsT=eqmat[:],
                                 rhs=probcT[:, j:j + 1], start=True,
                                 stop=True)
                nc.vector.tensor_copy(out=vals[:, j:j + 1], in_=psdup[:])

            if batched_indirect:
                g_all = _pt(wrk, [L, BL], F32, "g_all")
                nc.gpsimd.indirect_dma_start(
                    out=g_all[:], out_offset=None, in_=rowflat,
                    in_offset=bass.IndirectOffsetOnAxis(ap=soff[:], axis=0))
                vsum = _pt(wrk, [L, BL], F32, "vsum")
                nc.vector.tensor_add(out=vsum[:], in0=g_all[:], in1=vals[:])
                nc.gpsimd.indirect_dma_start(
                    out=rowflat, out_offset=bass.IndirectOffsetOnAxis(
                        ap=soff[:], axis=0),
                    in_=vsum[:], in_offset=None)
            else:
                for j in range(BL):
                    g_j = encj_p.tile([L, 1], F32, name="g_j", tag="g_j",
                                      bufs=2)
                    nc.gpsimd.indirect_dma_start(
                        out=g_j[:], out_offset=None, in_=rowflat,
                        in_offset=bass.IndirectOffsetOnAxis(
                            ap=seqiT[:, j:j + 1], axis=0),
                        element_offset=j * MV)
                    v_j = encj_p.tile([L, 1], F32, name="v_j", tag="v_j",
                                      bufs=2)
                    nc.vector.tensor_add(out=v_j[:], in0=g_j[:],
                                         in1=vals[:, j:j + 1])
                    nc.gpsimd.indirect_dma_start(
                        out=rowflat, out_offset=bass.IndirectOffsetOnAxis(
                            ap=seqiT[:, j:j + 1], axis=0),
                        in_=v_j[:], in_offset=None, element_offset=j * MV)

            c2t = _pt(wrk, [BL, 1], F32, "c2t")
            nc.vector.memset(c2t[:], 1e-9)
            nc.sync.dma_start(out=rowb[0:BL, 2:3], in_=c2t[:])

            # ============ log + store =====================================
            for k in range(8):
                nc.sync.dma_start(
                    out=sgl[k * BL:(k + 1) * BL, :],
                    in_=rowb[0:BL, k * FOLD:(k + 1) * FOLD])
            nc.scalar.activation(out=sgl[:], in_=sgl[:],
                                 func=mybir.ActivationFunctionType.Ln)
            for k in range(8):
                nc.sync.dma_start(
                    out=out1_d[0:BL, k * FOLD:(k + 1) * FOLD],
                    in_=sgl[k * BL:(k + 1) * BL, :])

    return nc


# ------------------------------------------------------------------
# host side
# ------------------------------------------------------------------
_NC_CACHE = {}


def _get_nc():
    if "nc" not in _NC_CACHE:
        _NC_CACHE["nc"] = build_nc()
    return _NC_CACHE["nc"]


def prepare_in_maps(inputs):
    f = lambda x: np.ascontiguousarray(np.asarray(x, dtype=np.float32))
    f8 = lambda x: np.ascontiguousarray(
        np.asarray(x, dtype=np.float32).astype(ml_dtypes.float8_e4m3))
    i32 = lambda x: np.ascontiguousarray(np.asarray(x).astype(np.int32))

    enc = f(inputs["encoder_outputs"])          # [64, 128, 1024]
    h0 = f(inputs["hidden"])[0]                 # [64, 1024]
    emb = f(inputs["emb"])                      # [32000, 512]
    attn_W = f(inputs["attn_W"])                # [128, 1536]
    attn_b = f(inputs["attn_b"])                # [128]
    comb_W = f(inputs["comb_W"])                # [1024, 1536]
    comb_b = f(inputs["comb_b"])                # [1024]
    W_ih = f(inputs["W_ih"])                    # [3072, 2048]
    W_hh = f(inputs["W_hh"])                    # [3072, 1024]
    b_ih = f(inputs["b_ih"])                    # [3072]
    b_hh = f(inputs["b_hh"])                    # [3072]
    Wo_W = f(inputs["Wo_W"])                    # [32000, 1024]
    Wo_b = f(inputs["Wo_b"])                    # [32000]
    Wc_W = f(inputs["Wc_W"])                    # [1024, 1024]
    Wc_b = f(inputs["Wc_b"])                    # [1024]
    tok = i32(inputs["input_tok"]).reshape(B, 1)
    seq = i32(inputs["input_seq"])              # [64, 128]
    pre = f(inputs["pre_prob"])                 # [64, 128]

    hT = np.ascontiguousarray(h0.T)             # [1024, 64]
    wcT = np.ascontiguousarray(Wc_W.T)          # [1024, 1024]
    wcb = np.ascontiguousarray(Wc_b.reshape(8, HC).T)   # [128, 8]
    ident = np.eye(128, dtype=np.float32)
    p_idx = np.arange(B)
    bc64 = (p_idx[None, :] % BL == np.arange(BL)[:, None]).astype(np.float32)
    selM = np.ascontiguousarray(bc64.T)
    onesf = np.ones((1, 128), np.float32)
    onesb = np.ones((1, B), np.float32).astype(ml_dtypes.float8_e4m3)
    attnWT_f = np.ascontiguousarray(attn_W.T)   # [1536, 128]

    in_maps = []
    for c in range(NCORES):
        bs = slice(c * BL, (c + 1) * BL)
        hs = slice(c * HC, (c + 1) * HC)
        vs = slice(c * VC, (c + 1) * VC)
        selB = np.zeros((B, BL), np.float32)
        selB[np.arange(c * BL, (c + 1) * BL), np.arange(BL)] = 1.0
        wihT3 = np.concatenate(
            [W_ih[g * H + c * HC:g * H + (c + 1) * HC, :].T
             for g in range(3)], axis=1).astype(ml_dtypes.bfloat16)
        whhT3 = np.ascontiguousarray(np.concatenate(
            [W_hh[g * H + c * HC:g * H + (c + 1) * HC, :].T
             for g in range(3)], axis=1))
        bih = np.stack([b_ih[g * H + c * HC:g * H + (c + 1) * HC]
                        .reshape(HC, 1) for g in range(3)])
        bhh = np.stack([b_hh[g * H + c * HC:g * H + (c + 1) * HC]
                        .reshape(HC, 1) for g in range(3)])
        awc = np.ascontiguousarray(
            np.concatenate([attnWT_f, comb_W[hs].T], axis=1))  # [1536, 256]
        in_maps.append({
            "enc_own": np.ascontiguousarray(enc[bs]),
            "encP_own": np.ascontiguousarray(
                enc[bs].transpose(2, 0, 1).reshape(H, BL * L)),
            "wcT": wcT,
            "wcb": wcb,
            "woT_own": f8(Wo_W[vs].T),
            "wob_own": f8(Wo_b[vs].reshape(1, VC)),
            "wihT3_own": np.ascontiguousarray(wihT3),
            "whhT3_own": whhT3,
            "bih_own": bih,
            "bhh_own": bhh,
            "awc_pack": awc,
            "combb_own": comb_b[hs].reshape(HC, 1).copy(),
            "attnb": attn_b.reshape(1, L).copy(),
            "hT": hT,
            "hT_own": np.ascontiguousarray(hT[hs]),
            "hT_owncols": np.ascontiguousarray(hT[:, bs]),
            "emb": emb,
            "tok64": tok,
            "tok_own": np.ascontiguousarray(tok[bs]),
            "seq_own": np.ascontiguousarray(seq[bs]),
            "pre_own": np.ascontiguousarray(pre[bs]),
            "ident": ident,
            "bc64": bc64,
            "selM": selM,
            "selB": selB,
            "onesf": onesf,
            "onesb": onesb,
        })
    return in_maps


def assemble(results):
    out1 = np.concatenate([results[c]["out1_own"] for c in range(NCORES)], 0)
    attnw = np.concatenate([results[c]["attnw_own"] for c in range(NCORES)], 0)
    probc = np.concatenate([results[c]["probc_own"] for c in range(NCORES)], 0)
    hnew = np.concatenate(
        [results[c]["hnewT_own"].T for c in range(NCORES)], 1)[None]
    return (out1.astype(np.float32), hnew.astype(np.float32),
            attnw.astype(np.float32), probc.astype(np.float32))


def run_spmd(in_maps, trace=False):
    from concourse.bass_utils import run_bass_kernel_spmd
    nc = _get_nc()
    if not nc.is_finalized():
        nc.finalize()   # runs Bacc register allocation before serialization
    return run_bass_kernel_spmd(nc, in_maps, list(range(NCORES)), trace=trace)


def kernel(**inputs):
    in_maps = prepare_in_maps(inputs)
    res = run_spmd(in_maps)
    return assemble(res.results)


# revision 29
# speedup vs baseline: 1.3691x; 1.0125x over previous
"""Trainium2 Bass kernel for nn_Decoder (pointer-generator style decoder step).

Strategy (8 NeuronCores, SPMD — identical program, per-core data):
  - Batch data-parallel: core c owns batches 8c..8c+8 (enc slice, sel/attn,
    score_c, final joint softmax + copy-scatter + log).
  - Tensor-parallel GRU/comb over hidden chunks (core c owns H rows
    128c..128c+128), with tiny AllGathers for gru_in / h_new assembly.
  - Tensor-parallel vocab projection: core c owns Wo rows 4000c..4000(c+1);
    an AllToAll redistributes score_g so each core holds full-vocab rows for
    its own batches.
  - Precision: f32 on every path feeding score_c (logits reach +-30) and the
    h_new state; bf16 for small-magnitude weight matmuls (W_ih, comb); fp8
    for Wo (vocab scores only matter in log-domain).
  - DMA-instruction count is the scarce resource (HWDGE issue ~0.5us/inst):
    inputs are host-packed for big contiguous loads, bulk loads issue on the
    scalar ring, latency-critical loads on the sync ring.
"""

import sys

import numpy as np

sys.path.insert(0, "/opt/trn_rl_repo")

import ml_dtypes  # noqa: E402

import concourse.bass as bass  # noqa: E402
import concourse.mybir as mybir  # noqa: E402
import concourse.tile as tile  # noqa: E402
from concourse import bacc  # noqa: E402

B = 64          # batch
L = 128         # max len
H = 1024        # hidden
E = 512         # embed
V = 32000       # vocab
MV = 33000      # max vocab (padded output)
NCORES = 8
BL = B // NCORES      # batches per core = 8
VC = V // NCORES      # vocab per core = 4000
HC = H // NCORES      # hidden chunk = 128
KV = 1536             # H + E
FOLD = MV // NCORES   # 4125, fold width for the log pass

F32 = mybir.dt.float32
BF16 = mybir.dt.bfloat16
F8 = mybir.dt.float8e4
I32 = mybir.dt.int32

RG = [list(range(NCORES))]


def _pt(pool, shape, dt, name):
    return pool.tile(shape, dt, name=name, tag=name)


def build_nc(batched_indirect=False):
    nc = bacc.Bacc("TRN2", target_bir_lowering=False, debug=False,
                   num_devices=NCORES)

    # ---------------- DRAM I/O ----------------
    enc_d = nc.dram_tensor("enc_own", [BL, L, H], F32, kind="ExternalInput")
    encP_d = nc.dram_tensor("encP_own", [H, BL * L], F32, kind="ExternalInput")
    wcT_d = nc.dram_tensor("wcT", [H, H], F32, kind="ExternalInput")
    wcb_d = nc.dram_tensor("wcb", [HC, NCORES], F32, kind="ExternalInput")
    woT_d = nc.dram_tensor("woT_own", [H, VC], F8, kind="ExternalInput")
    wob_d = nc.dram_tensor("wob_own", [1, VC], F8, kind="ExternalInput")
    wihT_d = nc.dram_tensor("wihT3_own", [2 * H, 3 * HC], BF16,
                            kind="ExternalInput")
    whhT_d = nc.dram_tensor("whhT3_own", [H, 3 * HC], F32,
                            kind="ExternalInput")
    bih_d = nc.dram_tensor("bih_own", [3, HC, 1], F32, kind="ExternalInput")
    bhh_d = nc.dram_tensor("bhh_own", [3, HC, 1], F32, kind="ExternalInput")
    # columns: [attnWT (128) | combT chunk (128)] packed by K row, f32
    awc_d = nc.dram_tensor("awc_pack", [KV, 2 * L], F32, kind="ExternalInput")
    combb_d = nc.dram_tensor("combb_own", [HC, 1], F32, kind="ExternalInput")
    attnb_d = nc.dram_tensor("attnb", [1, L], F32, kind="ExternalInput")
    hT_d = nc.dram_tensor("hT", [H, B], F32, kind="ExternalInput")
    hTown_d = nc.dram_tensor("hT_own", [HC, B], F32, kind="ExternalInput")
    hTcols_d = nc.dram_tensor("hT_owncols", [H, BL], F32, kind="ExternalInput")
    emb_d = nc.dram_tensor("emb", [V, E], F32, kind="ExternalInput")
    tok64_d = nc.dram_tensor("tok64", [B, 1], I32, kind="ExternalInput")
    tokown_d = nc.dram_tensor("tok_own", [BL, 1], I32, kind="ExternalInput")
    seq_d = nc.dram_tensor("seq_own", [BL, L], I32, kind="ExternalInput")
    pre_d = nc.dram_tensor("pre_own", [BL, L], F32, kind="ExternalInput")
    ident_d = nc.dram_tensor("ident", [128, 128], F32, kind="ExternalInput")
    bc64_d = nc.dram_tensor("bc64", [BL, B], F32, kind="ExternalInput")
    selM_d = nc.dram_tensor("selM", [B, BL], F32, kind="ExternalInput")
    selB_d = nc.dram_tensor("selB", [B, BL], F32, kind="ExternalInput")
    onesf_d = nc.dram_tensor("onesf", [1, 128], F32, kind="ExternalInput")
    onesb_d = nc.dram_tensor("onesb", [1, B], F8, kind="ExternalInput")

    out1_d = nc.dram_tensor("out1_own", [BL, MV], F32, kind="ExternalOutput")
    attnw_d = nc.dram_tensor("attnw_own", [BL, L], F32, kind="ExternalOutput")
    probc_d = nc.dram_tensor("probc_own", [BL, L], F32, kind="ExternalOutput")
    hnewT_d = nc.dram_tensor("hnewT_own", [HC, B], F32, kind="ExternalOutput")

    with tile.TileContext(nc) as tc:
        with (
            tc.tile_pool(name="big", bufs=1) as big,
            tc.tile_pool(name="wrk", bufs=1) as wrk,
            tc.tile_pool(name="encj", bufs=2) as encj_p,
            tc.tile_pool(name="wct", bufs=4) as wct_p,
            tc.tile_pool(name="ps_t", bufs=2, space="PSUM") as ps_t,
            tc.tile_pool(name="ps_s2", bufs=2, space="PSUM") as ps_s2,
            tc.tile_pool(name="ps_gate", bufs=2, space="PSUM") as ps_gate,
            tc.tile_pool(name="dram", bufs=1, space="DRAM") as dram,
        ):
            # ============ phase 0: small latency-critical loads (sync) ====
            ident = _pt(wrk, [128, 128], F32, "ident")
            nc.sync.dma_start(out=ident[:], in_=ident_d[:])
            tok64 = _pt(wrk, [B, 1], I32, "tok64")
            nc.sync.dma_start(out=tok64[:], in_=tok64_d[:])
            tokown = _pt(wrk, [BL, 1], I32, "tokown")
            nc.sync.dma_start(out=tokown[:], in_=tokown_d[:])
            seqi = _pt(wrk, [BL, L], I32, "seqi")
            nc.sync.dma_start(out=seqi[:], in_=seq_d[:])
            pre = _pt(wrk, [BL, L], F32, "pre")
            nc.sync.dma_start(out=pre[:], in_=pre_d[:])

            def load_rows(dram_ap, rows, cols, dt, pool, name, eng):
                n = rows // 128
                ts = []
                for k in range(n):
                    t = _pt(pool, [128, cols], dt, f"{name}{k}")
                    eng.dma_start(out=t[:],
                                  in_=dram_ap[k * 128:(k + 1) * 128, :])
                    ts.append(t)
                return ts

            # attn weights + comb weights in one packed load (sync)
            awc = load_rows(awc_d[:], KV, 2 * L, F32, big, "awc", nc.sync)
            attnWT = [t[:, 0:L] for t in awc]
            hTcols = load_rows(hTcols_d[:], H, BL, F32, big, "hTcols",
                               nc.sync)
            attnb = _pt(wrk, [1, L], F32, "attnb")
            nc.sync.dma_start(out=attnb[:], in_=attnb_d[:])
            onesf = _pt(wrk, [1, 128], F32, "onesf")
            nc.sync.dma_start(out=onesf[:], in_=onesf_d[:])

            # ============ bulk loads on the scalar HWDGE ring =============
            woT = load_rows(woT_d[:], H, VC, F8, big, "woT", nc.scalar)
            encTp = load_rows(encP_d[:], H, BL * L, F32, big, "encTp",
                              nc.scalar)
            wihT3 = load_rows(wihT_d[:], 2 * H, 3 * HC, BF16, big, "wihT3",
                              nc.scalar)
            whhT3 = load_rows(whhT_d[:], H, 3 * HC, F32, big, "whhT3",
                              nc.scalar)
            hT = load_rows(hT_d[:], H, B, F32, big, "hT", nc.scalar)
            bc64 = _pt(wrk, [BL, B], F32, "bc64")
            nc.scalar.dma_start(out=bc64[:], in_=bc64_d[:])
            selM = _pt(wrk, [B, BL], F32, "selM")
            nc.scalar.dma_start(out=selM[:], in_=selM_d[:])
            selB = _pt(wrk, [B, BL], F32, "selB")
            nc.scalar.dma_start(out=selB[:], in_=selB_d[:])
            onesb = _pt(wrk, [1, B], F8, "onesb")
            nc.scalar.dma_start(out=onesb[:], in_=onesb_d[:])
            wcb = _pt(wrk, [HC, NCORES], F32, "wcb")
            nc.scalar.dma_start(out=wcb[:], in_=wcb_d[:])
            combb = _pt(wrk, [HC, 1], F32, "combb")
            nc.scalar.dma_start(out=combb[:], in_=combb_d[:])
            wob = _pt(wrk, [1, VC], F8, "wob")
            nc.scalar.dma_start(out=wob[:], in_=wob_d[:])
            hTown = _pt(wrk, [HC, B], F32, "hTown")
            nc.scalar.dma_start(out=hTown[:], in_=hTown_d[:])
            bih = [_pt(wrk, [HC, 1], F32, f"bih{g}") for g in range(3)]
            bhh = [_pt(wrk, [HC, 1], F32, f"bhh{g}") for g in range(3)]
            for g in range(3):
                nc.scalar.dma_start(out=bih[g][:], in_=bih_d[g])
                nc.scalar.dma_start(out=bhh[g][:], in_=bhh_d[g])

            # ============ helpers =========================================
            identb = _pt(wrk, [128, 128], BF16, "identb")
            nc.vector.tensor_copy(out=identb[:], in_=ident[:])

            def pe_T(in_ap, pin, pout, name, out_dt=F32):
                idt = ident if in_ap.dtype == F32 else identb
                ps = ps_t.tile([128, 128], in_ap.dtype, name=f"psT_{name}",
                               tag="psT")
                nc.tensor.transpose(out=ps[:pout, :pin], in_=in_ap,
                                    identity=idt[:pin, :pin])
                sb = _pt(wrk, [pout, pin], out_dt, f"T_{name}")
                nc.vector.tensor_copy(out=sb[:], in_=ps[:pout, :pin])
                return sb

            # ============ embedding gathers ===============================
            emb64 = _pt(big, [B, E], F32, "emb64")
            nc.gpsimd.indirect_dma_start(
                out=emb64[:], out_offset=None, in_=emb_d[:],
                in_offset=bass.IndirectOffsetOnAxis(ap=tok64[:, :1], axis=0))
            embown = _pt(wrk, [BL, E], F32, "embown")
            nc.gpsimd.indirect_dma_start(
                out=embown[:], out_offset=None, in_=emb_d[:],
                in_offset=bass.IndirectOffsetOnAxis(ap=tokown[:, :1], axis=0))

            embT64 = [pe_T(emb64[:, k * 128:(k + 1) * 128], B, 128,
                           f"embT64_{k}", out_dt=BF16) for k in range(4)]
            embTown = [pe_T(embown[:, k * 128:(k + 1) * 128], BL, 128,
                            f"embTown_{k}") for k in range(4)]

            # ============ attention scores (own batches) ==================
            attn_lhs = embTown + [hTcols[k] for k in range(8)]
            ps_a = ps_t.tile([BL, L], F32, name="ps_a", tag="psT")
            for k in range(12):
                nc.tensor.matmul(out=ps_a[:], lhsT=attn_lhs[k][:],
                                 rhs=attnWT[k], start=(k == 0), stop=False)
            nc.tensor.matmul(out=ps_a[:], lhsT=onesf[:1, :BL], rhs=attnb[:],
                             start=False, stop=True)

            namax = _pt(wrk, [BL, 1], F32, "namax")
            nc.vector.reduce_max(out=namax[:], in_=ps_a[:],
                                 axis=mybir.AxisListType.X, negate=True)
            asum = _pt(wrk, [BL, 1], F32, "asum")
            nc.vector.memset(asum[:], 0.0)
            aexp = _pt(wrk, [BL, L], F32, "aexp")
            nc.scalar.activation(out=aexp[:], in_=ps_a[:],
                                 func=mybir.ActivationFunctionType.Exp,
                                 bias=namax[:, :1], accum_out=asum[:, :1])
            arec = _pt(wrk, [BL, 1], F32, "arec")
            nc.vector.reciprocal(out=arec[:], in_=asum[:])
            attnw = _pt(wrk, [BL, L], F32, "attnw")
            nc.vector.tensor_scalar_mul(attnw[:], aexp[:], arec[:, :1])
            nc.sync.dma_start(out=attnw_d[:], in_=attnw[:])
            attnwT = pe_T(attnw[:], BL, L, "attnwT")

            # ============ selective-read mask =============================
            tokf = _pt(wrk, [BL, 1], F32, "tokf")
            nc.vector.tensor_copy(out=tokf[:], in_=tokown[:])
            seqf = _pt(wrk, [BL, L], F32, "seqf")
            nc.vector.tensor_copy(out=seqf[:], in_=seqi[:])
            eqm = _pt(wrk, [BL, L], F32, "eqm")
            nc.vector.tensor_scalar(eqm[:], seqf[:], tokf[:, :1], None,
                                    op0=mybir.AluOpType.is_equal)
            selv = _pt(wrk, [BL, L], F32, "selv")
            nc.vector.tensor_mul(out=selv[:], in0=eqm[:], in1=pre[:])
            selT = pe_T(selv[:], BL, L, "selT")

            # ============ per-batch einsums over enc ======================
            cc1_in = _pt(dram, [BL, 2 * H], BF16, "cc1_in")
            for j in range(BL):
                row = encj_p.tile([1, 2 * H], BF16, name="ccrow", tag="ccrow",
                                  bufs=2)
                for h in range(2):
                    sl = slice(h * 512, (h + 1) * 512)
                    encb = encj_p.tile([L, 512], F32, name="encb", tag="encb")
                    nc.sync.dma_start(out=encb[:], in_=enc_d[j][:, sl])
                    enci = encj_p.tile([L, 512], I32, name="enci", tag="enci",
                                       bufs=2)
                    nc.vector.tensor_copy(out=enci[:], in_=encb[:])
                    enct = enci[:].bitcast(F32)
                    nc.vector.tensor_copy(out=enct, in_=enci[:])
                    psA = ps_s2.tile([1, 512], F32, name="psA", tag="ps2")
                    nc.tensor.matmul(out=psA[:], lhsT=attnwT[:, j:j + 1],
                                     rhs=encb[:], start=True, stop=True)
                    nc.vector.tensor_copy(out=row[:, sl], in_=psA[:])
                    psS = ps_s2.tile([1, 512], F32, name="psS", tag="ps2")
                    nc.tensor.matmul(out=psS[:], lhsT=selT[:, j:j + 1],
                                     rhs=enct, start=True, stop=True,
                                     skip_group_check=True)
                    nc.vector.tensor_copy(
                        out=row[:, H + h * 512:H + (h + 1) * 512], in_=psS[:])
                nc.sync.dma_start(out=cc1_in[j:j + 1, :], in_=row[:])

            # ============ C1T early: tanh(enc @ WcT + b) -> DRAM ==========
            t1_d = _pt(dram, [8, 128, BL * L], F32, "t1_d")
            for hop in range(4):          # pairs of ho chunks
                wcts = []
                for hi in range(8):
                    w = wct_p.tile([128, 256], F32, name="wct", tag="wct",
                                   bufs=8)
                    nc.sync.dma_start(
                        out=w[:], in_=wcT_d[hi * 128:(hi + 1) * 128,
                                           hop * 256:(hop + 1) * 256])
                    wcts.append(w)
                for hoi in range(2):
                    ho = hop * 2 + hoi
                    t1 = wct_p.tile([128, BL * L], F32, name="t1", tag="t1",
                                    bufs=2)
                    for h in range(2):
                        psc = ps_s2.tile([128, 512], F32, name="psc",
                                         tag="ps2")
                        sl = slice(h * 512, (h + 1) * 512)
                        for hi in range(8):
                            nc.tensor.matmul(
                                out=psc[:],
                                lhsT=wcts[hi][:, hoi * 128:(hoi + 1) * 128],
                                rhs=encTp[hi][:, sl],
                                start=(hi == 0), stop=(hi == 7))
                        nc.scalar.activation(
                            out=t1[:, sl], in_=psc[:],
                            func=mybir.ActivationFunctionType.Tanh,
                            bias=wcb[:, ho:ho + 1])
                    nc.sync.dma_start(out=t1_d[ho], in_=t1[:])

            # ============ AllGather 1: [attn_applied | sel_reading] =======
            cc1_out = dram.tile([B, 2 * H], BF16, name="cc1_out",
                                addr_space="Shared")
            nc.gpsimd.collective_compute(
                "AllGather", mybir.AluOpType.bypass, replica_groups=RG,
                ins=[cc1_in[:].opt()], outs=[cc1_out[:].opt()])

            rows1 = _pt(big, [B, 2 * H], BF16, "rows1")
            nc.sync.dma_start(out=rows1[:], in_=cc1_out[:])
            attnappT = [pe_T(rows1[:, k * 128:(k + 1) * 128], B, 128,
                             f"attnappT{k}", out_dt=BF16) for k in range(8)]
            selrT = [pe_T(rows1[:, H + k * 128:H + (k + 1) * 128], B, 128,
                          f"selrT{k}", out_dt=BF16) for k in range(8)]

            # ============ comb (TP over H chunk) ==========================
            combTb = []
            for k in range(12):
                t = _pt(wrk, [128, L], BF16, f"combTb{k}")
                nc.vector.tensor_copy(out=t[:], in_=awc[k][:, L:2 * L])
                combTb.append(t)
            catT = embT64 + attnappT
            ps_o = ps_t.tile([HC, B], F32, name="ps_o", tag="psT")
            for k in range(12):
                nc.tensor.matmul(out=ps_o[:], lhsT=combTb[k][:, :HC],
                                 rhs=catT[k][:], start=(k == 0),
                                 stop=(k == 11))
            outT = _pt(wrk, [HC, B], BF16, "outT")
            nc.scalar.activation(out=outT[:], in_=ps_o[:],
                                 func=mybir.ActivationFunctionType.Relu,
                                 bias=combb[:, :1])

            cc2_in = _pt(dram, [HC, B], BF16, "cc2_in")
            nc.sync.dma_start(out=cc2_in[:], in_=outT[:])
            cc2_out = dram.tile([H, B], BF16, name="cc2_out",
                                addr_space="Shared")
            nc.gpsimd.collective_compute(
                "AllGather", mybir.AluOpType.bypass, replica_groups=RG,
                ins=[cc2_in[:].opt()], outs=[cc2_out[:].opt()])
            outTf = load_rows(cc2_out[:], H, B, BF16, big, "outTf", nc.sync)

            # ============ GRU (TP over H chunk) ===========================
            gruinT = outTf + selrT  # 16 bf16 tiles [128, 64]

            def gate_bf(g, ps, stop_last):
                for k in range(16):
                    nc.tensor.matmul(out=ps[:],
                                     lhsT=wihT3[k][:, g * HC:(g + 1) * HC],
                                     rhs=gruinT[k][:], start=(k == 0),
                                     stop=(stop_last and k == 15),
                                     skip_group_check=True)

            def gate_f32(g, ps, start, stop):
                for k in range(8):
                    nc.tensor.matmul(out=ps[:],
                                     lhsT=whhT3[k][:, g * HC:(g + 1) * HC],
                                     rhs=hT[k][:], start=(start and k == 0),
                                     stop=(stop and k == 7),
                                     skip_group_check=True)

            ps_hn = ps_gate.tile([HC, B], F32, name="ps_hn", tag="gate")
            gate_f32(2, ps_hn, True, True)
            hn_g = _pt(wrk, [HC, B], F32, "hn_g")
            nc.vector.tensor_scalar_add(hn_g[:], ps_hn[:], bhh[2][:, :1])

            ps_r = ps_gate.tile([HC, B], F32, name="ps_r", tag="gate")
            gate_bf(0, ps_r, False)
            gate_f32(0, ps_r, False, True)
            brz = _pt(wrk, [HC, 1], F32, "brz")
            nc.vector.tensor_add(out=brz[:], in0=bih[0][:], in1=bhh[0][:])
            r_g = _pt(wrk, [HC, B], F32, "r_g")
            nc.scalar.activation(out=r_g[:], in_=ps_r[:],
                                 func=mybir.ActivationFunctionType.Sigmoid,
                                 bias=brz[:, :1])

            ps_in = ps_gate.tile([HC, B], F32, name="ps_in", tag="gate")
            gate_bf(2, ps_in, True)
            ps_z = ps_gate.tile([HC, B], F32, name="ps_z", tag="gate")
            gate_bf(1, ps_z, False)
            gate_f32(1, ps_z, False, True)
            bzz = _pt(wrk, [HC, 1], F32, "bzz")
            nc.vector.tensor_add(out=bzz[:], in0=bih[1][:], in1=bhh[1][:])
            z_g = _pt(wrk, [HC, B], F32, "z_g")
            nc.scalar.activation(out=z_g[:], in_=ps_z[:],
                                 func=mybir.ActivationFunctionType.Sigmoid,
                                 bias=bzz[:, :1])

            rn = _pt(wrk, [HC, B], F32, "rn")
            nc.vector.tensor_mul(out=rn[:], in0=r_g[:], in1=hn_g[:])
            narg = _pt(wrk, [HC, B], F32, "narg")
            nc.vector.tensor_add(out=narg[:], in0=ps_in[:], in1=rn[:])
            n_g = _pt(wrk, [HC, B], F32, "n_g")
            nc.scalar.activation(out=n_g[:], in_=narg[:],
                                 func=mybir.ActivationFunctionType.Tanh,
                                 bias=bih[2][:, :1])
            dmn = _pt(wrk, [HC, B], F32, "dmn")
            nc.vector.tensor_sub(out=dmn[:], in0=hTown[:], in1=n_g[:])
            zd = _pt(wrk, [HC, B], F32, "zd")
            nc.vector.tensor_mul(out=zd[:], in0=z_g[:], in1=dmn[:])
            hnT_c = _pt(wrk, [HC, B], F32, "hnT_c")
            nc.vector.tensor_add(out=hnT_c[:], in0=n_g[:], in1=zd[:])
            nc.sync.dma_start(out=hnewT_d[:], in_=hnT_c[:])

            cc3_in = _pt(dram, [HC, B], F32, "cc3_in")
            nc.sync.dma_start(out=cc3_in[:], in_=hnT_c[:])
            cc3_out = dram.tile([H, B], F32, name="cc3_out",
                                addr_space="Shared")
            nc.gpsimd.collective_compute(
                "AllGather", mybir.AluOpType.bypass, replica_groups=RG,
                ins=[cc3_in[:].opt()], outs=[cc3_out[:].opt()])
            hnTf = load_rows(cc3_out[:], H, B, F32, big, "hnTf", nc.sync)

            # ============ score_g (fp8 TP vocab) + AllToAll ===============
            hnT8 = []
            for k in range(8):
                t = _pt(wrk, [128, B], F8, f"hnT8{k}")
                nc.vector.tensor_copy(out=t[:], in_=hnTf[k][:])
                hnT8.append(t)
            cc4_in = _pt(dram, [B, VC], F32, "cc4_in")
            NG = 500
            for v in range(VC // NG):
                psg = ps_s2.tile([B, NG], F32, name="psg", tag="ps2")
                sl = slice(v * NG, (v + 1) * NG)
                for k in range(8):
                    nc.tensor.matmul(out=psg[:], lhsT=hnT8[k][:],
                                     rhs=woT[k][:, sl], start=(k == 0),
                                     stop=False)
                nc.tensor.matmul(out=psg[:], lhsT=onesb[:1, :B],
                                 rhs=wob[:, sl], start=False, stop=True)
                sbg = encj_p.tile([B, NG], F32, name="sbg", tag="sbg",
                                  bufs=2)
                nc.vector.tensor_copy(out=sbg[:], in_=psg[:])
                nc.sync.dma_start(out=cc4_in[:, sl], in_=sbg[:])
            cc4_out = dram.tile([B, VC], F32, name="cc4_out")
            nc.gpsimd.collective_compute(
                "AllToAll", mybir.AluOpType.bypass, replica_groups=RG,
                ins=[cc4_in[:].opt()], outs=[cc4_out[:].opt()])

            # ============ own-batch h_new columns =========================
            hrows = _pt(wrk, [B, H], F32, "hrows")
            for k in range(8):
                psh = ps_t.tile([128, 128], F32, name="psh", tag="psT")
                nc.tensor.transpose(out=psh[:B, :128], in_=hnTf[k][:],
                                    identity=ident[:, :])
                nc.vector.tensor_copy(out=hrows[:, k * 128:(k + 1) * 128],
                                      in_=psh[:B, :128])
            hrown = _pt(wrk, [BL, H], F32, "hrown")
            for h in range(2):
                psr = ps_s2.tile([128, 512], F32, name="psr", tag="ps2")
                nc.tensor.matmul(out=psr[:BL, :], lhsT=selB[:],
                                 rhs=hrows[:, h * 512:(h + 1) * 512],
                                 start=True, stop=True)
                nc.vector.tensor_copy(out=hrown[:, h * 512:(h + 1) * 512],
                                      in_=psr[:BL, :])
            hnTcol = [pe_T(hrown[:, k * 128:(k + 1) * 128], BL, 128,
                           f"hnTcol{k}") for k in range(8)]

            # ============ score_c einsum from spilled t1 ==================
            scacc = _pt(wrk, [L, BL], F32, "scacc")
            for ho in range(8):
                t1r = wct_p.tile([128, BL * L], F32, name="t1r", tag="t1",
                                 bufs=2)
                nc.sync.dma_start(out=t1r[:], in_=t1_d[ho])
                ps_sc = ps_gate.tile([L, BL], F32, name="ps_sc", tag="scT")
                for j in range(BL):
                    nc.tensor.matmul(out=ps_sc[:, j:j + 1],
                                     lhsT=t1r[:, j * L:(j + 1) * L],
                                     rhs=hnTcol[ho][:, j:j + 1],
                                     start=(j == 0), stop=(j == BL - 1),
                                     skip_group_check=True)
                if ho == 0:
                    nc.vector.tensor_copy(out=scacc[:], in_=ps_sc[:])
                else:
                    nc.vector.tensor_add(out=scacc[:], in0=scacc[:],
                                         in1=ps_sc[:])
            scoreC = pe_T(scacc[:], L, BL, "scoreC")

            # ============ joint softmax ===================================
            # sgl partition p = 8*chunk + own-batch (batch = p % 8)
            sgl = _pt(big, [B, FOLD], F32, "sgl")
            sg = sgl[:, :VC]
            nc.sync.dma_start(out=sg, in_=cc4_out[:])

            m2n = _pt(wrk, [BL, 1], F32, "m2n")
            nc.vector.reduce_max(out=m2n[:], in_=scoreC[:],
                                 axis=mybir.AxisListType.X, negate=True)
            s2 = _pt(wrk, [BL, 1], F32, "s2")
            nc.vector.memset(s2[:], 0.0)
            ec = _pt(wrk, [BL, L], F32, "ec")
            nc.scalar.activation(out=ec[:], in_=scoreC[:],
                                 func=mybir.ActivationFunctionType.Exp,
                                 bias=m2n[:, :1], accum_out=s2[:, :1])

            psb = ps_t.tile([B, 1], F32, name="psb", tag="psT")
            nc.tensor.matmul(out=psb[:], lhsT=bc64[:], rhs=m2n[:],
                             start=True, stop=True)
            m2n64 = _pt(wrk, [B, 1], F32, "m2n64")
            nc.vector.tensor_copy(out=m2n64[:], in_=psb[:])

            s1 = _pt(wrk, [B, 1], F32, "s1")
            nc.vector.memset(s1[:], 0.0)
            nc.scalar.activation(out=sg, in_=sg,
                                 func=mybir.ActivationFunctionType.Exp,
                                 bias=m2n64[:, :1], accum_out=s1[:, :1])

            psv = ps_t.tile([BL, 1], F32, name="psv", tag="psT")
            nc.tensor.matmul(out=psv[:], lhsT=selM[:], rhs=s1[:],
                             start=True, stop=True)
            stot = _pt(wrk, [BL, 1], F32, "stot")
            nc.vector.tensor_add(out=stot[:], in0=s2[:], in1=psv[:])
            rec = _pt(wrk, [BL, 1], F32, "rec")
            nc.vector.reciprocal(out=rec[:], in_=stot[:])

            probc = _pt(wrk, [BL, L], F32, "probc")
            nc.vector.tensor_scalar_mul(probc[:], ec[:], rec[:, :1])
            nc.sync.dma_start(out=probc_d[:], in_=probc[:])

            psb2 = ps_t.tile([B, 1], F32, name="psb2", tag="psT")
            nc.tensor.matmul(out=psb2[:], lhsT=bc64[:], rhs=rec[:],
                             start=True, stop=True)
            rec64 = _pt(wrk, [B, 1], F32, "rec64")
            nc.vector.tensor_copy(out=rec64[:], in_=psb2[:])
            nc.vector.tensor_scalar_mul(sg, sg, rec64[:, :1])

            # ============ scatter-add of copy probs =======================
            rowb = _pt(dram, [BL, MV], F32, "rowb")
            rowflat = rowb[:].rearrange("a b -> (a b)")[:, None]
            for k in range(8):
                nc.sync.dma_start(
                    out=rowb[0:BL, k * VC:(k + 1) * VC],
                    in_=sgl[k * BL:(k + 1) * BL, :VC])
            padt = _pt(wrk, [BL, MV - V], F32, "padt")
            nc.vector.memset(padt[:], 1e-9)
            nc.sync.dma_start(out=rowb[0:BL, V:MV], in_=padt[:])

            seqfT = pe_T(seqf[:], BL, L, "seqfT")
            seqiT = _pt(wrk, [L, BL], I32, "seqiT")
            nc.vector.tensor_copy(out=seqiT[:], in_=seqfT[:])
            ioff = _pt(wrk, [L, BL], I32, "ioff")
            nc.gpsimd.iota(ioff[:], pattern=[[1, BL]], base=0,
                           channel_multiplier=0)
            ioffm = _pt(wrk, [L, BL], I32, "ioffm")
            nc.vector.tensor_scalar(ioffm[:], ioff[:], MV, None,
                                    op0=mybir.AluOpType.mult)
            soff = _pt(wrk, [L, BL], I32, "soff")
            nc.vector.tensor_add(out=soff[:], in0=seqiT[:], in1=ioffm[:])
            probcT = pe_T(probc[:], BL, L, "probcT")

            # duplicate-index accumulation (selmat per batch)
            vals = _pt(wrk, [L, BL], F32, "vals")
            for j in range(BL):
                sri = encj_p.tile([1, L], I32, name="sri", tag="sri", bufs=2)
                nc.sync.dma_start(out=sri[:], in_=seq_d[j:j + 1, :])
                srf = encj_p.tile([1, L], F32, name="srf", tag="srf", bufs=2)
                nc.vector.tensor_copy(out=srf[:], in_=sri[:])
                psrep = ps_t.tile([L, L], F32, name="psrep", tag="psT")
                nc.tensor.matmul(out=psrep[:], lhsT=onesf[:1, :L],
                                 rhs=srf[:], start=True, stop=True)
                eqmat = encj_p.tile([L, L], F32, name="eqmat", tag="eqmat",
                                    bufs=1)
                nc.vector.tensor_scalar(eqmat[:], psrep[:],
                                        seqfT[:, j:j + 1], None,
                                        op0=mybir.AluOpType.is_equal)
                psdup = ps_t.tile([L, 1], F32, name="psdup", tag="psT")
                nc.tensor.matmul(out=psdup[:], lhsT=eqmat[:],
                                 rhs=probcT[:, j:j + 1], start=True,
                                 stop=True)
                nc.vector.tensor_copy(out=vals[:, j:j + 1], in_=psdup[:])

            if batched_indirect:
                g_all = _pt(wrk, [L, BL], F32, "g_all")
                nc.gpsimd.indirect_dma_start(
                    out=g_all[:], out_offset=None, in_=rowflat,
                    in_offset=bass.IndirectOffsetOnAxis(ap=soff[:], axis=0))
                vsum = _pt(wrk, [L, BL], F32, "vsum")
                nc.vector.tensor_add(out=vsum[:], in0=g_all[:], in1=vals[:])
                nc.gpsimd.indirect_dma_start(
                    out=rowflat, out_offset=bass.IndirectOffsetOnAxis(
                        ap=soff[:], axis=0),
                    in_=vsum[:], in_offset=None)
            else:
                for j in range(BL):
                    g_j = encj_p.tile([L, 1], F32, name="g_j", tag="g_j",
                                      bufs=2)
                    nc.gpsimd.indirect_dma_start(
                        out=g_j[:], out_offset=None, in_=rowflat,
                        in_offset=bass.IndirectOffsetOnAxis(
                            ap=seqiT[:, j:j + 1], axis=0),
                        element_offset=j * MV)
                    v_j = encj_p.tile([L, 1], F32, name="v_j", tag="v_j",
                                      bufs=2)
                    nc.vector.tensor_add(out=v_j[:], in0=g_j[:],
                                         in1=vals[:, j:j + 1])
                    nc.gpsimd.indirect_dma_start(
                        out=rowflat, out_offset=bass.IndirectOffsetOnAxis(
                            ap=seqiT[:, j:j + 1], axis=0),
                        in_=v_j[:], in_offset=None, element_offset=j * MV)

            c2t = _pt(wrk, [BL, 1], F32, "c2t")
            nc.vector.memset(c2t[:], 1e-9)
            nc.sync.dma_start(out=rowb[0:BL, 2:3], in_=c2t[:])

            # ============ log + store =====================================
            for k in range(8):
                nc.sync.dma_start(
                    out=sgl[k * BL:(k + 1) * BL, :],
                    in_=rowb[0:BL, k * FOLD:(k + 1) * FOLD])
            nc.scalar.activation(out=sgl[:], in_=sgl[:],
                                 func=mybir.ActivationFunctionType.Ln)
            for k in range(8):
                nc.sync.dma_start(
                    out=out1_d[0:BL, k * FOLD:(k + 1) * FOLD],
                    in_=sgl[k * BL:(k + 1) * BL, :])

    return nc


# ------------------------------------------------------------------
# host side
# ------------------------------------------------------------------
_NC_CACHE = {}


def _get_nc():
    if "nc" not in _NC_CACHE:
        _NC_CACHE["nc"] = build_nc()
    return _NC_CACHE["nc"]


def prepare_in_maps(inputs):
    f = lambda x: np.ascontiguousarray(np.asarray(x, dtype=np.float32))
    f8 = lambda x: np.ascontiguousarray(
        np.asarray(x, dtype=np.float32).astype(ml_dtypes.float8_e4m3))
    i32 = lambda x: np.ascontiguousarray(np.asarray(x).astype(np.int32))

    enc = f(inputs["encoder_outputs"])          # [64, 128, 1024]
    h0 = f(inputs["hidden"])[0]                 # [64, 1024]
    emb = f(inputs["emb"])                      # [32000, 512]
    attn_W = f(inputs["attn_W"])                # [128, 1536]
    attn_b = f(inputs["attn_b"])                # [128]
    comb_W = f(inputs["comb_W"])                # [1024, 1536]
    comb_b = f(inputs["comb_b"])                # [1024]
    W_ih = f(inputs["W_ih"])                    # [3072, 2048]
    W_hh = f(inputs["W_hh"])                    # [3072, 1024]
    b_ih = f(inputs["b_ih"])                    # [3072]
    b_hh = f(inputs["b_hh"])                    # [3072]
    Wo_W = f(inputs["Wo_W"])                    # [32000, 1024]
    Wo_b = f(inputs["Wo_b"])                    # [32000]
    Wc_W = f(inputs["Wc_W"])                    # [1024, 1024]
    Wc_b = f(inputs["Wc_b"])                    # [1024]
    tok = i32(inputs["input_tok"]).reshape(B, 1)
    seq = i32(inputs["input_seq"])              # [64, 128]
    pre = f(inputs["pre_prob"])                 # [64, 128]

    hT = np.ascontiguousarray(h0.T)             # [1024, 64]
    wcT = np.ascontiguousarray(Wc_W.T)          # [1024, 1024]
    wcb = np.ascontiguousarray(Wc_b.reshape(8, HC).T)   # [128, 8]
    ident = np.eye(128, dtype=np.float32)
    p_idx = np.arange(B)
    bc64 = (p_idx[None, :] % BL == np.arange(BL)[:, None]).astype(np.float32)
    selM = np.ascontiguousarray(bc64.T)
    onesf = np.ones((1, 128), np.float32)
    onesb = np.ones((1, B), np.float32).astype(ml_dtypes.float8_e4m3)
    attnWT_f = np.ascontiguousarray(attn_W.T)   # [1536, 128]

    in_maps = []
    for c in range(NCORES):
        bs = slice(c * BL, (c + 1) * BL)
        hs = slice(c * HC, (c + 1) * HC)
        vs = slice(c * VC, (c + 1) * VC)
        selB = np.zeros((B, BL), np.float32)
        selB[np.arange(c * BL, (c + 1) * BL), np.arange(BL)] = 1.0
        wihT3 = np.concatenate(
            [W_ih[g * H + c * HC:g * H + (c + 1) * HC, :].T
             for g in range(3)], axis=1).astype(ml_dtypes.bfloat16)
        whhT3 = np.ascontiguousarray(np.concatenate(
            [W_hh[g * H + c * HC:g * H + (c + 1) * HC, :].T
             for g in range(3)], axis=1))
        bih = np.stack([b_ih[g * H + c * HC:g * H + (c + 1) * HC]
                        .reshape(HC, 1) for g in range(3)])
        bhh = np.stack([b_hh[g * H + c * HC:g * H + (c + 1) * HC]
                        .reshape(HC, 1) for g in range(3)])
        awc = np.ascontiguousarray(
            np.concatenate([attnWT_f, comb_W[hs].T], axis=1))  # [1536, 256]
        in_maps.append({
            "enc_own": np.ascontiguousarray(enc[bs]),
            "encP_own": np.ascontiguousarray(
                enc[bs].transpose(2, 0, 1).reshape(H, BL * L)),
            "wcT": wcT,
            "wcb": wcb,
            "woT_own": f8(Wo_W[vs].T),
            "wob_own": f8(Wo_b[vs].reshape(1, VC)),
            "wihT3_own": np.ascontiguousarray(wihT3),
            "whhT3_own": whhT3,
            "bih_own": bih,
            "bhh_own": bhh,
            "awc_pack": awc,
            "combb_own": comb_b[hs].reshape(HC, 1).copy(),
            "attnb": attn_b.reshape(1, L).copy(),
            "hT": hT,
            "hT_own": np.ascontiguousarray(hT[hs]),
            "hT_owncols": np.ascontiguousarray(hT[:, bs]),
            "emb": emb,
            "tok64": tok,
            "tok_own": np.ascontiguousarray(tok[bs]),
            "seq_own": np.ascontiguousarray(seq[bs]),
            "pre_own": np.ascontiguousarray(pre[bs]),
            "ident": ident,
            "bc64": bc64,
            "selM": selM,
            "selB": selB,
            "onesf": onesf,
            "onesb": onesb,
        })
    return in_maps


def assemble(results):
    out1 = np.concatenate([results[c]["out1_own"] for c in range(NCORES)], 0)
    attnw = np.concatenate([results[c]["attnw_own"] for c in range(NCORES)], 0)
    probc = np.concatenate([results[c]["probc_own"] for c in range(NCORES)], 0)
    hnew = np.concatenate(
        [results[c]["hnewT_own"].T for c in range(NCORES)], 1)[None]
    return (out1.astype(np.float32), hnew.astype(np.float32),
            attnw.astype(np.float32), probc.astype(np.float32))


def run_spmd(in_maps, trace=False):
    from concourse.bass_utils import run_bass_kernel_spmd
    nc = _get_nc()
    if not nc.is_finalized():
        nc.finalize()   # runs Bacc register allocation before serialization
    return run_bass_kernel_spmd(nc, in_maps, list(range(NCORES)), trace=trace)


def kernel(**inputs):
    in_maps = prepare_in_maps(inputs)
    res = run_spmd(in_maps)
    return assemble(res.results)
